# revision 27
# baseline (speedup 1.0000x reference)
"""Multi-head causal self-attention (B=2, L=2048, D=1024, H=16) on 8 TRN2
NeuronCores.

Sharding: core c handles batch b = c // 4 and head group g = c % 4 (4 heads,
i.e. a 256-wide slice of the QKV output dim and the matching 256 rows of
Wo^T).  Each core computes a full (L, D) partial of the output projection;
the host sums the 4 partials per batch (bf16) and adds bo.

v3 structure:
 * Host pre-transposes + pre-casts x / W slices to bf16 in the exact on-chip
   layout (XT [128,8,2048], W*T [128,8,256], WoT [128,2,1024]) -- no on-chip
   transposes or casts at all.
 * Scores are row-packed: KTp keeps the natural head-pair layout (head 2hj
   on partitions 0-63, head 2hj+1 on 64-127); per k-tile TWO concurrent
   K=64 matmuls (tile_position row groups 0 / 64) write the two halves of a
   2-bank PSUM pair -> score stream time halves vs zero-padded K=128.
 * exp on ACT per k-tile over the [128, 2x512] pair (diag tiles use a
   strided AP to skip fully-masked leading columns of both halves).
 * AV lags one k-tile behind exp so the PE never waits on ACT; causal mask
   via Pool multiply with a triangle (off the DVE).
 * qk/v projections of block b+1 and the output projection of block b-1 are
   generators, consumed 2 matmuls per attention iteration (the PE slack
   under the 1146ns exp pace), leftovers drained at pair/block ends.
 * normalize = PE ones-broadcast of the denominator row + DVE
   reciprocal_approx_fast + multiply, deferred one head-pair.
"""

import sys

for _p in ("/opt/trn_rl_repo", "/root/.axon_site/_ro/trn_rl_repo"):
    if _p not in sys.path:
        sys.path.append(_p)

from contextlib import ExitStack

import numpy as np
import ml_dtypes

import concourse.bass as bass
import concourse.tile as tile
from concourse import bacc, mybir
from concourse.bass_utils import run_bass_kernel_spmd
from concourse.masks import make_identity

F32 = mybir.dt.float32
F16 = mybir.dt.float16
BF16 = mybir.dt.bfloat16

B, L, D, H = 2, 2048, 1024, 16
DK = D // H  # 64
NCORES = 8
GH = 4  # heads per core
C = GH * DK  # 256: per-core slice of the qkv/head dim
QT_TILES = L // 512  # 4
DCH = D // 128  # 8


class Weave:
    """FIFO of generators; take(n) advances up to n emission steps."""

    def __init__(self, *gens):
        self.gens = list(gens)

    def push(self, gen):
        self.gens.append(gen)

    def take(self, n):
        while n > 0 and self.gens:
            try:
                next(self.gens[0])
                n -= 1
            except StopIteration:
                self.gens.pop(0)

    def drain(self):
        while self.gens:
            try:
                next(self.gens[0])
            except StopIteration:
                self.gens.pop(0)

    def finish_first(self):
        if self.gens:
            for _ in self.gens[0]:
                pass
            self.gens.pop(0)


def _build_program():
    nc = bacc.Bacc("TRN2", target_bir_lowering=False, debug=False, num_devices=NCORES)

    xt_d = nc.dram_tensor("xt", [128, DCH, L], BF16, kind="ExternalInput").ap()
    wq_d = nc.dram_tensor("wqt", [128, 2, DCH, 128], BF16, kind="ExternalInput").ap()
    wk_d = nc.dram_tensor("wkt", [128, 2, DCH, 128], BF16, kind="ExternalInput").ap()
    wv_d = nc.dram_tensor("wvt", [128, 2, DCH, 128], BF16, kind="ExternalInput").ap()
    wo_d = nc.dram_tensor("wot", [128, 2, D], BF16, kind="ExternalInput").ap()
    b6_d = nc.dram_tensor("b6", [128, 6], F32, kind="ExternalInput").ap()
    out_d = nc.dram_tensor("out", [L, D], BF16, kind="ExternalOutput").ap()

    with tile.TileContext(nc) as tc, ExitStack() as ctx:
        pool = ctx.enter_context(tc.tile_pool(name="persist", bufs=1))
        psum = ctx.enter_context(tc.tile_pool(name="psum", bufs=2, space="PSUM"))
        psum_o = ctx.enter_context(tc.tile_pool(name="psum_o", bufs=2, space="PSUM"))
        psum2 = ctx.enter_context(tc.tile_pool(name="psum2", bufs=2, space="PSUM"))
        cp = ctx.enter_context(tc.tile_pool(name="copies", bufs=3))
        yp = ctx.enter_context(tc.tile_pool(name="youts", bufs=2))

        XT = pool.tile([128, DCH, L], BF16)
        WT = {}
        for name in ("q", "k", "v"):
            WT[name] = pool.tile([128, 2, DCH, 128], BF16, name=f"W{name}T")
        WoT = pool.tile([128, 2, D], BF16)
        b6_sb = pool.tile([128, 6], F32)

        # ---- prologue DMAs.  The ~2.5 MB that gates block-0's projections
        # (wk/wq j=0 halves, x block 0, wv) is split evenly across the two
        # HWDGE rings in consumption order so the k/q/v matmul streams start
        # as chunks land; everything later-needed follows.
        for dci in range(0, 4):
            nc.sync.dma_start(WT["k"][:, 0, 2 * dci : 2 * dci + 2], wk_d[:, 0, 2 * dci : 2 * dci + 2])
            nc.sync.dma_start(XT[:, dci, 0:512], xt_d[:, dci, 0:512])
        nc.scalar.dma_start(WT["q"][:, 0], wq_d[:, 0])
        nc.scalar.dma_start(b6_sb[:], b6_d[:])
        for dci in range(4, 8):
            nc.scalar.dma_start(XT[:, dci, 0:512], xt_d[:, dci, 0:512])
        nc.sync.dma_start(WT["v"][:, 0], wv_d[:, 0])
        nc.sync.dma_start(WT["v"][:, 1], wv_d[:, 1])
        nc.scalar.dma_start(WT["k"][:, 1], wk_d[:, 1])
        nc.scalar.dma_start(WT["q"][:, 1], wq_d[:, 1])
        for dci in range(0, 4):
            nc.sync.dma_start(XT[:, dci, 512:1024], xt_d[:, dci, 512:1024])
        for dci in range(4, 8):
            nc.scalar.dma_start(XT[:, dci, 512:1024], xt_d[:, dci, 512:1024])
        nc.sync.dma_start(WoT[:], wo_d[:])
        for dci in range(DCH):
            nc.sync.dma_start(XT[:, dci, 1024:2048], xt_d[:, dci, 1024:2048])

        BIAS_COL = {"q": 0, "k": 2, "v": 4}

        ones_f32 = pool.tile([1, 128], F32)
        nc.gpsimd.memset(ones_f32[:], 1.0)
        ones_r = pool.tile([1, 128], mybir.dt.float32r)
        nc.vector.tensor_copy(ones_r[:], ones_f32[:])
        ident16 = pool.tile([128, 128], F16)
        make_identity(nc, ident16)
        tri_mask = pool.tile([128, 128], F16)
        nc.gpsimd.memset(tri_mask[:], 1.0)
        nc.gpsimd.affine_select(
            out=tri_mask[:],
            in_=tri_mask[:],
            pattern=[[1, 128]],
            compare_op=mybir.AluOpType.is_ge,
            fill=0.0,
            base=0,
            channel_multiplier=-1,
        )

        QTs = [pool.tile([128, 2, 512], F16, name=f"QT{g}") for g in range(4)]
        # k^T zero-padded per head to K=128 rows: the PE HAM clock gate only
        # un-throttles when matmuls stream all 128 partitions (64-row packed
        # score tiles measured to re-throttle the clock mid-kernel).
        KTzs = [pool.tile([128, GH, 512], F16, name=f"KTz{g}") for g in range(4)]
        Vp16 = [pool.tile([128, 4, GH, DK + 1], F16, name=f"Vp16_{g}") for g in range(4)]
        OTs = [pool.tile([128, 2, 512], BF16, name=f"OT{g}") for g in range(4)]

        for g in range(4):
            for h in range(GH):
                zp = 64 - 64 * (h % 2)
                nc.gpsimd.memset(KTzs[g][zp : zp + 64, h, :], 0.0)
        for g in range(4):
            nc.gpsimd.memset(Vp16[g][:, :, :, DK], 1.0)

        with nc.allow_low_precision(reason="bf16/f16 matmul inputs"):

            def kq_gen(blk, j):
                # k then q projection of one 128-col slice; yields per PE op.
                for name in ("k", "q"):
                    ps = psum.tile([128, 512], F32, tag="ps")
                    for dci in range(DCH):
                        nc.tensor.matmul(
                            ps[:],
                            lhsT=WT[name][:, j, dci, :],
                            rhs=XT[:, dci, blk * 512 : (blk + 1) * 512],
                            start=(dci == 0),
                            stop=(dci == DCH - 1),
                        )
                        yield
                    if name == "q":
                        nc.vector.tensor_tensor(
                            QTs[blk][:, j, :],
                            ps[:],
                            b6_sb[:, BIAS_COL["q"] + j, None].to_broadcast(
                                (128, 512)
                            ),
                            mybir.AluOpType.add,
                        )
                    else:
                        for half in range(2):
                            hp = 64 * half
                            nc.vector.tensor_tensor(
                                KTzs[blk][hp : hp + 64, 2 * j + half, :],
                                ps[hp : hp + 64, :],
                                b6_sb[
                                    hp : hp + 64, BIAS_COL["k"] + j, None
                                ].to_broadcast((64, 512)),
                                mybir.AluOpType.add,
                            )
                    yield

            def v_gen(blk):
                vt = cp.tile([128, 2, 512], F16, tag="vt", bufs=2)
                for ch in range(2):
                    ps = psum.tile([128, 512], F32, tag="ps")
                    for dci in range(DCH):
                        nc.tensor.matmul(
                            ps[:],
                            lhsT=WT["v"][:, ch, dci, :],
                            rhs=XT[:, dci, blk * 512 : (blk + 1) * 512],
                            start=(dci == 0),
                            stop=(dci == DCH - 1),
                        )
                        yield
                    nc.vector.tensor_tensor(
                        vt[:, ch, :],
                        ps[:],
                        b6_sb[:, BIAS_COL["v"] + ch, None].to_broadcast((128, 512)),
                        mybir.AluOpType.add,
                    )
                    yield
                for lsub in range(4):
                    pv = psum.tile([128, 256], F16, tag="ps")
                    for ch in range(2):
                        nc.tensor.matmul(
                            pv[:, ch * 128 : (ch + 1) * 128],
                            lhsT=vt[:, ch, lsub * 128 : (lsub + 1) * 128],
                            rhs=ident16[:],
                            is_transpose=True,
                            start=(ch == 0),
                            stop=(ch == 1),
                        )
                        yield
                    nc.vector.tensor_copy(
                        Vp16[blk][:, lsub, :, 0:DK],
                        pv[:].rearrange("p (h d) -> p h d", h=GH),
                    )
                    yield

            def chain(*gens):
                for g in gens:
                    yield from g

            def proj_gen(blk):
                # j=0 k/q first, then v, then j=1: attention on the block can
                # begin (heads 0/1) before the j=1 half exists.
                return chain(kq_gen(blk, 0), v_gen(blk), kq_gen(blk, 1))

            def normalize(h, qt, pso):
                hj, hp = h // 2, 64 * (h % 2)
                den_r = cp.tile([1, 512], mybir.dt.float32r, tag="den", bufs=2)
                nc.vector.tensor_copy(den_r[:], pso[64:65, :])
                psb = psum.tile([128, 512], F32, tag="ps")
                nc.tensor.matmul(
                    psb[:64], lhsT=ones_r[:, 0:64], rhs=den_r[:], start=True, stop=True
                )
                rb = cp.tile([64, 512], F32, tag="rb", bufs=2)
                nc.vector.reciprocal_approx_fast(rb[:], psb[:64])
                nc.vector.tensor_tensor(
                    OTs[qt][hp : hp + 64, hj, :],
                    pso[:64],
                    rb[:],
                    mybir.AluOpType.mult,
                )

            def normalize_pair(hj, qt, pso_e, pso_o):
                normalize(2 * hj, qt, pso_e)
                normalize(2 * hj + 1, qt, pso_o)

            def outproj_gen(qt512):
                for sub in range(4):
                    for e in range(2):
                        psy = psum.tile([128, 512], F32, tag="ps")
                        for cj in range(2):
                            nc.tensor.matmul(
                                psy[:],
                                lhsT=OTs[qt512][:, cj, sub * 128 : (sub + 1) * 128],
                                rhs=WoT[:, cj, e * 512 : (e + 1) * 512],
                                start=(cj == 0),
                                stop=(cj == 1),
                            )
                            yield
                        y_sb = yp.tile([128, 512], BF16, tag="y")
                        nc.vector.tensor_copy(y_sb[:], psy[:])
                        q0 = qt512 * 512 + sub * 128
                        nc.sync.dma_start(
                            out_d[q0 : q0 + 128, e * 512 : (e + 1) * 512], y_sb[:]
                        )
                        yield

            y3 = {}
            y8 = ctx.enter_context(tc.tile_pool(name="ytail", bufs=8))

            def outproj_half_gen(qt512, cj, subs=(0, 1, 2, 3)):
                # last-block outproj split by contraction half: cj=0 runs
                # during the final attention pair, cj=1 + accumulate + DMA is
                # all that trails the final normalize.
                for sub in subs:
                    for e in range(2):
                        idx = sub * 2 + e
                        psy = psum.tile([128, 512], F32, tag="ps")
                        nc.tensor.matmul(
                            psy[:],
                            lhsT=OTs[qt512][:, cj, sub * 128 : (sub + 1) * 128],
                            rhs=WoT[:, cj, e * 512 : (e + 1) * 512],
                            start=True,
                            stop=True,
                        )
                        yield
                        if cj == 0:
                            y_sb = y8.tile([128, 512], F32, tag="y8")
                            nc.vector.tensor_copy(y_sb[:], psy[:])
                            y3[idx] = y_sb
                        else:
                            y_sb = y3[idx]
                            yb = y8.tile([128, 512], BF16, tag="yb8")
                            nc.vector.tensor_tensor(
                                yb[:], y_sb[:], psy[:], mybir.AluOpType.add
                            )
                            q0 = qt512 * 512 + sub * 128
                            eng = nc.sync if idx % 2 == 0 else nc.scalar
                            eng.dma_start(
                                out_d[q0 : q0 + 128, e * 512 : (e + 1) * 512], yb[:]
                            )
                        yield

            pending = None

            def attn(qt, weave):
                nonlocal pending
                n_kt = 4 * qt + 4
                for hj in range(2):
                    if qt == 0 and hj == 1:
                        # block-0 only: finish j=1 projections before heads 2/3
                        weave.finish_first()
                    pso_e = psum_o.tile([128, 512], F32, tag="pso")
                    pso_o = psum_o.tile([128, 512], F32, tag="pso")
                    avq = []

                    def emit_av(kt, p_sb, d0):
                        g, ksub = kt // 4, kt % 4
                        for i, pso in ((0, pso_e), (1, pso_o)):
                            nc.tensor.matmul(
                                pso[:65, d0:],
                                lhsT=Vp16[g][:, ksub, 2 * hj + i, :],
                                rhs=p_sb[:, i * 512 + d0 : (i + 1) * 512],
                                start=(kt == 0),
                                stop=(kt == n_kt - 1),
                                skip_group_check=True,
                            )

                    for kt in range(n_kt):
                        g, ksub = kt // 4, kt % 4
                        pss = psum2.tile([128, 1024], F32, tag="ps2")
                        d0 = max(0, (kt - 4 * qt) * 128)
                        for i in range(2):  # the head pair, full-K=128 each
                            nc.tensor.matmul(
                                pss[:, i * 512 + d0 : (i + 1) * 512],
                                lhsT=KTzs[g][
                                    :, 2 * hj + i, ksub * 128 : (ksub + 1) * 128
                                ],
                                rhs=QTs[qt][:, hj, d0:],
                                start=True,
                                stop=True,
                            )
                        p_sb = cp.tile([128, 1024], F16, tag="p", bufs=5)
                        if d0:
                            nc.scalar.activation(
                                p_sb[:].rearrange("p (i q) -> p i q", i=2)[:, :, d0:],
                                pss[:].rearrange("p (i q) -> p i q", i=2)[:, :, d0:],
                                mybir.ActivationFunctionType.Exp,
                                scale=0.125,
                            )
                        else:
                            nc.scalar.activation(
                                p_sb[:],
                                pss[:],
                                mybir.ActivationFunctionType.Exp,
                                scale=0.125,
                            )
                        if kt >= 4 * qt:  # diagonal tile: causal mask, Pool
                            for i in range(2):
                                nc.gpsimd.tensor_tensor(
                                    p_sb[:, i * 512 + d0 : i * 512 + d0 + 128],
                                    p_sb[:, i * 512 + d0 : i * 512 + d0 + 128],
                                    tri_mask[:],
                                    mybir.AluOpType.mult,
                                )
                        avq.append((kt, p_sb, d0))
                        # AV lags three k-tiles behind exp: by emission time
                        # its exp AND Pool masks have both finished
                        if len(avq) > 3:
                            emit_av(*avq.pop(0))
                        elif kt == 0 and pending is not None:
                            normalize_pair(*pending)
                            pending = None
                            if qt == QT_TILES - 1 and hj == 1:
                                weave.push(outproj_half_gen(qt, 0))
                        weave.take(2 if len(avq) <= 3 else 1)
                    for a in avq:
                        emit_av(*a)
                    pending = (hj, qt, pso_e, pso_o)

            # ==== block 0: j=0 k/q + v eagerly, then attention on heads 0/1
            # with the j=1 projections (then block 1's) woven in.
            for _ in chain(kq_gen(0, 0), v_gen(0)):
                pass
            for blk in range(QT_TILES):
                gens = []
                if blk == 0:
                    gens.append(kq_gen(0, 1))
                if blk > 0:
                    gens.append(outproj_gen(blk - 1))
                if blk < QT_TILES - 1:
                    gens.append(proj_gen(blk + 1))
                weave = Weave(*gens)
                attn(blk, weave)
                if blk < QT_TILES - 1:
                    weave.drain()
            # final normalize first, then drain leftovers: the PE chews the
            # remaining output-projection matmuls while the normalize's DVE
            # chain runs (keeps HAM warm into the tail)
            normalize_pair(*pending)
            weave.drain()
            for _ in outproj_half_gen(QT_TILES - 1, 1):
                pass

    nc.compile()
    return nc


_NC_CACHE = None


def _get_program():
    global _NC_CACHE
    if _NC_CACHE is None:
        _NC_CACHE = _build_program()
    return _NC_CACHE


def _run(in_maps, trace=False, **kw):
    nc = _get_program()
    return run_bass_kernel_spmd(nc, in_maps, list(range(NCORES)), trace=trace, **kw)


def _chunked_T(a, nch):
    """[R, Cc] -> [128, nch, Cc] with [p, i, c] = a[i*128+p, c], bf16."""
    r, c = a.shape
    assert r == nch * 128
    return np.ascontiguousarray(
        a.reshape(nch, 128, c).transpose(1, 0, 2)
    ).astype(ml_dtypes.bfloat16)


def _jmajor_T(a):
    """[D, C=256] -> [128, 2, DCH, 128] with [p, j, i, c] = a[i*128+p, j*128+c]."""
    d, c = a.shape
    assert d == DCH * 128 and c == 256
    return np.ascontiguousarray(
        a.reshape(DCH, 128, 2, 128).transpose(1, 2, 0, 3)
    ).astype(ml_dtypes.bfloat16)


def _make_in_maps(x, Wq, bq, Wk, bk, Wv, bv, Wo, bo):
    x = np.asarray(x, dtype=np.float32)
    Wq, Wk, Wv, Wo = (np.asarray(w, dtype=np.float32) for w in (Wq, Wk, Wv, Wo))
    bq, bk, bv = (np.asarray(b, dtype=np.float32) for b in (bq, bk, bv))
    in_maps = []
    xts = [_chunked_T(x[b].T, DCH) for b in range(B)]
    for core in range(NCORES):
        b, g = divmod(core, 4)
        s = slice(g * C, (g + 1) * C)
        b6 = np.stack(
            [
                bq[s][0:128], bq[s][128:256],
                bk[s][0:128], bk[s][128:256],
                bv[s][0:128], bv[s][128:256],
            ],
            axis=1,
        )
        in_maps.append(
            {
                "xt": xts[b],
                "wqt": _jmajor_T(Wq[s, :].T),
                "wkt": _jmajor_T(Wk[s, :].T),
                "wvt": _jmajor_T(Wv[s, :].T),
                "wot": _chunked_T(Wo[:, s].T, 2),
                "b6": np.ascontiguousarray(b6, dtype=np.float32),
            }
        )
    return in_maps


def kernel(x, Wq, bq, Wk, bk, Wv, bv, Wo, bo, _trace=False, _trace_out=None, _tmpdir=None):
    in_maps = _make_in_maps(x, Wq, bq, Wk, bk, Wv, bv, Wo, bo)
    res = _run(in_maps, trace=_trace, tmpdir=_tmpdir)
    if _trace_out is not None:
        _trace_out.append(res)
    bo = np.asarray(bo, dtype=np.float32)
    out = np.empty((B, L, D), dtype=np.float32)
    for b in range(B):
        acc = res.results[4 * b]["out"].astype(np.float32)
        for g in range(1, 4):
            acc = acc + res.results[4 * b + g]["out"].astype(np.float32)
        out[b] = acc + bo[None, :]
    return out


# revision 28
# speedup vs baseline: 1.1598x; 1.1598x over previous
"""Multi-head causal self-attention (B=2, L=2048, D=1024, H=16) on 8 TRN2
NeuronCores.

Sharding: core c handles batch b = c // 4 and head group g = c % 4 (4 heads,
i.e. a 256-wide slice of the QKV output dim and the matching 256 rows of
Wo^T).  Each core computes a full (L, D) partial of the output projection;
the host sums the 4 partials per batch (bf16) and adds bo.

v3 structure:
 * Host pre-transposes + pre-casts x / W slices to bf16 in the exact on-chip
   layout (XT [128,8,2048], W*T [128,8,256], WoT [128,2,1024]) -- no on-chip
   transposes or casts at all.
 * Scores are row-packed: KTp keeps the natural head-pair layout (head 2hj
   on partitions 0-63, head 2hj+1 on 64-127); per k-tile TWO concurrent
   K=64 matmuls (tile_position row groups 0 / 64) write the two halves of a
   2-bank PSUM pair -> score stream time halves vs zero-padded K=128.
 * exp on ACT per k-tile over the [128, 2x512] pair (diag tiles use a
   strided AP to skip fully-masked leading columns of both halves).
 * AV lags one k-tile behind exp so the PE never waits on ACT; causal mask
   via Pool multiply with a triangle (off the DVE).
 * qk/v projections of block b+1 and the output projection of block b-1 are
   generators, consumed 2 matmuls per attention iteration (the PE slack
   under the 1146ns exp pace), leftovers drained at pair/block ends.
 * normalize = PE ones-broadcast of the denominator row + DVE
   reciprocal_approx_fast + multiply, deferred one head-pair.
"""

import sys

for _p in ("/opt/trn_rl_repo", "/root/.axon_site/_ro/trn_rl_repo"):
    if _p not in sys.path:
        sys.path.append(_p)

from contextlib import ExitStack

import numpy as np
import ml_dtypes

import concourse.bass as bass
import concourse.tile as tile
from concourse import bacc, mybir
from concourse.bass_utils import run_bass_kernel_spmd
from concourse.masks import make_identity

F32 = mybir.dt.float32
F16 = mybir.dt.float16
BF16 = mybir.dt.bfloat16

B, L, D, H = 2, 2048, 1024, 16
DK = D // H  # 64
NCORES = 8
GH = 4  # heads per core
C = GH * DK  # 256: per-core slice of the qkv/head dim
QT_TILES = L // 512  # 4
DCH = D // 128  # 8


class Weave:
    """FIFO of generators; take(n) advances up to n emission steps."""

    def __init__(self, *gens):
        self.gens = list(gens)

    def push(self, gen):
        self.gens.append(gen)

    def take(self, n):
        while n > 0 and self.gens:
            try:
                next(self.gens[0])
                n -= 1
            except StopIteration:
                self.gens.pop(0)

    def drain(self):
        while self.gens:
            try:
                next(self.gens[0])
            except StopIteration:
                self.gens.pop(0)

    def finish_first(self):
        if self.gens:
            for _ in self.gens[0]:
                pass
            self.gens.pop(0)


def _build_program():
    nc = bacc.Bacc("TRN2", target_bir_lowering=False, debug=False, num_devices=NCORES)

    xt_d = nc.dram_tensor("xt", [128, DCH, L], BF16, kind="ExternalInput").ap()
    wq_d = nc.dram_tensor("wqt", [128, 2, DCH, 128], BF16, kind="ExternalInput").ap()
    wk_d = nc.dram_tensor("wkt", [128, 2, DCH, 128], BF16, kind="ExternalInput").ap()
    wv_d = nc.dram_tensor("wvt", [128, 2, DCH, 128], BF16, kind="ExternalInput").ap()
    wo_d = nc.dram_tensor("wot", [128, 2, D], BF16, kind="ExternalInput").ap()
    b6_d = nc.dram_tensor("b6", [128, 6], F32, kind="ExternalInput").ap()
    out_d = nc.dram_tensor("out", [L, D], BF16, kind="ExternalOutput").ap()

    with tile.TileContext(nc) as tc, ExitStack() as ctx:
        pool = ctx.enter_context(tc.tile_pool(name="persist", bufs=1))
        psum = ctx.enter_context(tc.tile_pool(name="psum", bufs=2, space="PSUM"))
        psum_o = ctx.enter_context(tc.tile_pool(name="psum_o", bufs=2, space="PSUM"))
        psum2 = ctx.enter_context(tc.tile_pool(name="psum2", bufs=2, space="PSUM"))
        cp = ctx.enter_context(tc.tile_pool(name="copies", bufs=3))
        yp = ctx.enter_context(tc.tile_pool(name="youts", bufs=2))

        XT = pool.tile([128, DCH, L], BF16)
        WT = {}
        for name in ("q", "k", "v"):
            WT[name] = pool.tile([128, 2, DCH, 128], BF16, name=f"W{name}T")
        WoT = pool.tile([128, 2, D], BF16)
        b6_sb = pool.tile([128, 6], F32)

        # ---- prologue DMAs.  The ~2.5 MB that gates block-0's projections
        # (wk/wq j=0 halves, x block 0, wv) is split evenly across the two
        # HWDGE rings in consumption order so the k/q/v matmul streams start
        # as chunks land; everything later-needed follows.
        for dci in range(0, 4):
            nc.sync.dma_start(WT["k"][:, 0, 2 * dci : 2 * dci + 2], wk_d[:, 0, 2 * dci : 2 * dci + 2])
            nc.sync.dma_start(XT[:, dci, 0:512], xt_d[:, dci, 0:512])
        nc.scalar.dma_start(WT["q"][:, 0], wq_d[:, 0])
        nc.scalar.dma_start(b6_sb[:], b6_d[:])
        for dci in range(4, 8):
            nc.scalar.dma_start(XT[:, dci, 0:512], xt_d[:, dci, 0:512])
        nc.sync.dma_start(WT["v"][:, 0], wv_d[:, 0])
        nc.sync.dma_start(WT["v"][:, 1], wv_d[:, 1])
        nc.scalar.dma_start(WT["k"][:, 1], wk_d[:, 1])
        nc.scalar.dma_start(WT["q"][:, 1], wq_d[:, 1])
        for dci in range(0, 4):
            nc.sync.dma_start(XT[:, dci, 512:1024], xt_d[:, dci, 512:1024])
        for dci in range(4, 8):
            nc.scalar.dma_start(XT[:, dci, 512:1024], xt_d[:, dci, 512:1024])
        nc.sync.dma_start(WoT[:], wo_d[:])
        for dci in range(DCH):
            nc.sync.dma_start(XT[:, dci, 1024:2048], xt_d[:, dci, 1024:2048])

        BIAS_COL = {"q": 0, "k": 2, "v": 4}

        ones_f32 = pool.tile([1, 128], F32)
        nc.gpsimd.memset(ones_f32[:], 1.0)
        ones_r = pool.tile([1, 128], mybir.dt.float32r)
        nc.vector.tensor_copy(ones_r[:], ones_f32[:])
        ident16 = pool.tile([128, 128], F16)
        make_identity(nc, ident16)
        tri_mask = pool.tile([128, 128], F16)
        nc.gpsimd.memset(tri_mask[:], 1.0)
        nc.gpsimd.affine_select(
            out=tri_mask[:],
            in_=tri_mask[:],
            pattern=[[1, 128]],
            compare_op=mybir.AluOpType.is_ge,
            fill=0.0,
            base=0,
            channel_multiplier=-1,
        )

        QTs = [pool.tile([128, 2, 512], F16, name=f"QT{g}") for g in range(4)]
        # k^T zero-padded per head to K=128 rows: the PE HAM clock gate only
        # un-throttles when matmuls stream all 128 partitions (64-row packed
        # score tiles measured to re-throttle the clock mid-kernel).
        KTzs = [pool.tile([128, GH, 512], F16, name=f"KTz{g}") for g in range(4)]
        Vp16 = [pool.tile([128, 4, GH, DK + 1], F16, name=f"Vp16_{g}") for g in range(4)]
        OTs = [pool.tile([128, 2, 512], BF16, name=f"OT{g}") for g in range(4)]

        for g in range(4):
            for h in range(GH):
                zp = 64 - 64 * (h % 2)
                nc.gpsimd.memset(KTzs[g][zp : zp + 64, h, :], 0.0)
        for g in range(4):
            nc.gpsimd.memset(Vp16[g][:, :, :, DK], 1.0)

        with nc.allow_low_precision(reason="bf16/f16 matmul inputs"):

            def kq_gen(blk, j):
                # k then q projection of one 128-col slice; yields per PE op.
                for name in ("k", "q"):
                    ps = psum.tile([128, 512], F32, tag="ps")
                    for dci in range(DCH):
                        nc.tensor.matmul(
                            ps[:],
                            lhsT=WT[name][:, j, dci, :],
                            rhs=XT[:, dci, blk * 512 : (blk + 1) * 512],
                            start=(dci == 0),
                            stop=(dci == DCH - 1),
                        )
                        yield
                    if name == "q":
                        nc.vector.tensor_tensor(
                            QTs[blk][:, j, :],
                            ps[:],
                            b6_sb[:, BIAS_COL["q"] + j, None].to_broadcast(
                                (128, 512)
                            ),
                            mybir.AluOpType.add,
                        )
                    else:
                        for half in range(2):
                            hp = 64 * half
                            nc.vector.tensor_tensor(
                                KTzs[blk][hp : hp + 64, 2 * j + half, :],
                                ps[hp : hp + 64, :],
                                b6_sb[
                                    hp : hp + 64, BIAS_COL["k"] + j, None
                                ].to_broadcast((64, 512)),
                                mybir.AluOpType.add,
                            )
                    yield

            def v_gen(blk):
                vt = cp.tile([128, 2, 512], F16, tag="vt", bufs=2)
                for ch in range(2):
                    ps = psum.tile([128, 512], F32, tag="ps")
                    for dci in range(DCH):
                        nc.tensor.matmul(
                            ps[:],
                            lhsT=WT["v"][:, ch, dci, :],
                            rhs=XT[:, dci, blk * 512 : (blk + 1) * 512],
                            start=(dci == 0),
                            stop=(dci == DCH - 1),
                        )
                        yield
                    nc.vector.tensor_tensor(
                        vt[:, ch, :],
                        ps[:],
                        b6_sb[:, BIAS_COL["v"] + ch, None].to_broadcast((128, 512)),
                        mybir.AluOpType.add,
                    )
                    yield
                for lsub in range(4):
                    pv = psum.tile([128, 256], F16, tag="ps")
                    for ch in range(2):
                        nc.tensor.matmul(
                            pv[:, ch * 128 : (ch + 1) * 128],
                            lhsT=vt[:, ch, lsub * 128 : (lsub + 1) * 128],
                            rhs=ident16[:],
                            is_transpose=True,
                            start=(ch == 0),
                            stop=(ch == 1),
                        )
                        yield
                    nc.vector.tensor_copy(
                        Vp16[blk][:, lsub, :, 0:DK],
                        pv[:].rearrange("p (h d) -> p h d", h=GH),
                    )
                    yield

            def chain(*gens):
                for g in gens:
                    yield from g

            def proj_gen(blk):
                # j=0 k/q first, then v, then j=1: attention on the block can
                # begin (heads 0/1) before the j=1 half exists.
                return chain(kq_gen(blk, 0), v_gen(blk), kq_gen(blk, 1))

            def normalize(h, qt, pso):
                hj, hp = h // 2, 64 * (h % 2)
                den_r = cp.tile([1, 512], mybir.dt.float32r, tag="den", bufs=2)
                nc.vector.tensor_copy(den_r[:], pso[64:65, :])
                psb = psum.tile([128, 512], F32, tag="ps")
                nc.tensor.matmul(
                    psb[:64], lhsT=ones_r[:, 0:64], rhs=den_r[:], start=True, stop=True
                )
                rb = cp.tile([64, 512], F32, tag="rb", bufs=2)
                nc.vector.reciprocal_approx_fast(rb[:], psb[:64])
                nc.vector.tensor_tensor(
                    OTs[qt][hp : hp + 64, hj, :],
                    pso[:64],
                    rb[:],
                    mybir.AluOpType.mult,
                )

            def normalize_pair(hj, qt, pso_e, pso_o):
                normalize(2 * hj, qt, pso_e)
                normalize(2 * hj + 1, qt, pso_o)

            def outproj_gen(qt512):
                for sub in range(4):
                    for e in range(2):
                        psy = psum.tile([128, 512], F32, tag="ps")
                        for cj in range(2):
                            nc.tensor.matmul(
                                psy[:],
                                lhsT=OTs[qt512][:, cj, sub * 128 : (sub + 1) * 128],
                                rhs=WoT[:, cj, e * 512 : (e + 1) * 512],
                                start=(cj == 0),
                                stop=(cj == 1),
                            )
                            yield
                        y_sb = yp.tile([128, 512], BF16, tag="y")
                        nc.vector.tensor_copy(y_sb[:], psy[:])
                        q0 = qt512 * 512 + sub * 128
                        nc.sync.dma_start(
                            out_d[q0 : q0 + 128, e * 512 : (e + 1) * 512], y_sb[:]
                        )
                        yield

            y3 = {}
            y8 = ctx.enter_context(tc.tile_pool(name="ytail", bufs=8))

            def outproj_half_gen(qt512, cj, subs=(0, 1, 2, 3)):
                # last-block outproj split by contraction half: cj=0 runs
                # during the final attention pair, cj=1 + accumulate + DMA is
                # all that trails the final normalize.
                for sub in subs:
                    for e in range(2):
                        idx = sub * 2 + e
                        psy = psum.tile([128, 512], F32, tag="ps")
                        nc.tensor.matmul(
                            psy[:],
                            lhsT=OTs[qt512][:, cj, sub * 128 : (sub + 1) * 128],
                            rhs=WoT[:, cj, e * 512 : (e + 1) * 512],
                            start=True,
                            stop=True,
                        )
                        yield
                        if cj == 0:
                            y_sb = y8.tile([128, 512], F32, tag="y8")
                            nc.vector.tensor_copy(y_sb[:], psy[:])
                            y3[idx] = y_sb
                        else:
                            y_sb = y3[idx]
                            yb = y8.tile([128, 512], BF16, tag="yb8")
                            nc.vector.tensor_tensor(
                                yb[:], y_sb[:], psy[:], mybir.AluOpType.add
                            )
                            q0 = qt512 * 512 + sub * 128
                            eng = nc.sync if idx % 2 == 0 else nc.scalar
                            eng.dma_start(
                                out_d[q0 : q0 + 128, e * 512 : (e + 1) * 512], yb[:]
                            )
                        yield

            pending = None

            def attn(qt, weave):
                nonlocal pending
                n_kt = 4 * qt + 4
                for hj in range(2):
                    if qt == 0 and hj == 1:
                        # block-0 only: finish j=1 projections before heads 2/3
                        weave.finish_first()
                    pso_e = psum_o.tile([128, 512], F32, tag="pso")
                    pso_o = psum_o.tile([128, 512], F32, tag="pso")
                    avq = []

                    def emit_av(kt, p_sb, d0):
                        g, ksub = kt // 4, kt % 4
                        for i, pso in ((0, pso_e), (1, pso_o)):
                            nc.tensor.matmul(
                                pso[:65, d0:],
                                lhsT=Vp16[g][:, ksub, 2 * hj + i, :],
                                rhs=p_sb[:, i * 512 + d0 : (i + 1) * 512],
                                start=(kt == 0),
                                stop=(kt == n_kt - 1),
                                skip_group_check=True,
                            )

                    for kt in range(n_kt):
                        g, ksub = kt // 4, kt % 4
                        pss = psum2.tile([128, 1024], F32, tag="ps2")
                        d0 = max(0, (kt - 4 * qt) * 128)
                        for i in range(2):  # the head pair, full-K=128 each
                            nc.tensor.matmul(
                                pss[:, i * 512 + d0 : (i + 1) * 512],
                                lhsT=KTzs[g][
                                    :, 2 * hj + i, ksub * 128 : (ksub + 1) * 128
                                ],
                                rhs=QTs[qt][:, hj, d0:],
                                start=True,
                                stop=True,
                            )
                        p_sb = cp.tile([128, 1024], F16, tag="p", bufs=5)
                        if d0:
                            nc.scalar.activation(
                                p_sb[:].rearrange("p (i q) -> p i q", i=2)[:, :, d0:],
                                pss[:].rearrange("p (i q) -> p i q", i=2)[:, :, d0:],
                                mybir.ActivationFunctionType.Exp,
                                scale=0.125,
                            )
                        else:
                            nc.scalar.activation(
                                p_sb[:],
                                pss[:],
                                mybir.ActivationFunctionType.Exp,
                                scale=0.125,
                            )
                        if kt >= 4 * qt:  # diagonal tile: causal mask, Pool
                            for i in range(2):
                                nc.gpsimd.tensor_tensor(
                                    p_sb[:, i * 512 + d0 : i * 512 + d0 + 128],
                                    p_sb[:, i * 512 + d0 : i * 512 + d0 + 128],
                                    tri_mask[:],
                                    mybir.AluOpType.mult,
                                )
                        avq.append((kt, p_sb, d0))
                        # AV lags two k-tiles behind exp: by emission time its
                        # exp AND Pool masks have both finished -> no PE wait
                        if len(avq) > 2:
                            emit_av(*avq.pop(0))
                        elif kt == 0 and pending is not None:
                            normalize_pair(*pending)
                            pending = None
                            if qt == QT_TILES - 1 and hj == 1:
                                weave.push(outproj_half_gen(qt, 0))
                        weave.take(2 if len(avq) <= 2 else 1)
                    for a in avq:
                        emit_av(*a)
                    pending = (hj, qt, pso_e, pso_o)

            # ==== block 0: j=0 k/q + v eagerly, then attention on heads 0/1
            # with the j=1 projections (then block 1's) woven in.
            for _ in chain(kq_gen(0, 0), v_gen(0)):
                pass
            for blk in range(QT_TILES):
                gens = []
                if blk == 0:
                    gens.append(kq_gen(0, 1))
                if blk > 0:
                    gens.append(outproj_gen(blk - 1))
                if blk < QT_TILES - 1:
                    gens.append(proj_gen(blk + 1))
                weave = Weave(*gens)
                attn(blk, weave)
                if blk < QT_TILES - 1:
                    weave.drain()
            # final normalize first, then drain leftovers: the PE chews the
            # remaining output-projection matmuls while the normalize's DVE
            # chain runs (keeps HAM warm into the tail)
            normalize_pair(*pending)
            weave.drain()
            for _ in outproj_half_gen(QT_TILES - 1, 1):
                pass

    nc.compile()
    return nc


_NC_CACHE = None


def _get_program():
    global _NC_CACHE
    if _NC_CACHE is None:
        _NC_CACHE = _build_program()
    return _NC_CACHE


def _run(in_maps, trace=False, **kw):
    nc = _get_program()
    return run_bass_kernel_spmd(nc, in_maps, list(range(NCORES)), trace=trace, **kw)


def _chunked_T(a, nch):
    """[R, Cc] -> [128, nch, Cc] with [p, i, c] = a[i*128+p, c], bf16."""
    r, c = a.shape
    assert r == nch * 128
    return np.ascontiguousarray(
        a.reshape(nch, 128, c).transpose(1, 0, 2)
    ).astype(ml_dtypes.bfloat16)


def _jmajor_T(a):
    """[D, C=256] -> [128, 2, DCH, 128] with [p, j, i, c] = a[i*128+p, j*128+c]."""
    d, c = a.shape
    assert d == DCH * 128 and c == 256
    return np.ascontiguousarray(
        a.reshape(DCH, 128, 2, 128).transpose(1, 2, 0, 3)
    ).astype(ml_dtypes.bfloat16)


def _make_in_maps(x, Wq, bq, Wk, bk, Wv, bv, Wo, bo):
    x = np.asarray(x, dtype=np.float32)
    Wq, Wk, Wv, Wo = (np.asarray(w, dtype=np.float32) for w in (Wq, Wk, Wv, Wo))
    bq, bk, bv = (np.asarray(b, dtype=np.float32) for b in (bq, bk, bv))
    in_maps = []
    xts = [_chunked_T(x[b].T, DCH) for b in range(B)]
    for core in range(NCORES):
        b, g = divmod(core, 4)
        s = slice(g * C, (g + 1) * C)
        b6 = np.stack(
            [
                bq[s][0:128], bq[s][128:256],
                bk[s][0:128], bk[s][128:256],
                bv[s][0:128], bv[s][128:256],
            ],
            axis=1,
        )
        in_maps.append(
            {
                "xt": xts[b],
                "wqt": _jmajor_T(Wq[s, :].T),
                "wkt": _jmajor_T(Wk[s, :].T),
                "wvt": _jmajor_T(Wv[s, :].T),
                "wot": _chunked_T(Wo[:, s].T, 2),
                "b6": np.ascontiguousarray(b6, dtype=np.float32),
            }
        )
    return in_maps


def kernel(x, Wq, bq, Wk, bk, Wv, bv, Wo, bo, _trace=False, _trace_out=None, _tmpdir=None):
    in_maps = _make_in_maps(x, Wq, bq, Wk, bk, Wv, bv, Wo, bo)
    res = _run(in_maps, trace=_trace, tmpdir=_tmpdir)
    if _trace_out is not None:
        _trace_out.append(res)
    bo = np.asarray(bo, dtype=np.float32)
    out = np.empty((B, L, D), dtype=np.float32)
    for b in range(B):
        acc = res.results[4 * b]["out"].astype(np.float32)
        for g in range(1, 4):
            acc = acc + res.results[4 * b + g]["out"].astype(np.float32)
        out[b] = acc + bo[None, :]
    return out


# revision 29
# speedup vs baseline: 1.1770x; 1.0148x over previous
"""Multi-head causal self-attention (B=2, L=2048, D=1024, H=16) on 8 TRN2
NeuronCores.

Sharding: core c handles batch b = c // 4 and head group g = c % 4 (4 heads,
i.e. a 256-wide slice of the QKV output dim and the matching 256 rows of
Wo^T).  Each core computes a full (L, D) partial of the output projection;
the host sums the 4 partials per batch (bf16) and adds bo.

v3 structure:
 * Host pre-transposes + pre-casts x / W slices to bf16 in the exact on-chip
   layout (XT [128,8,2048], W*T [128,8,256], WoT [128,2,1024]) -- no on-chip
   transposes or casts at all.
 * Scores are row-packed: KTp keeps the natural head-pair layout (head 2hj
   on partitions 0-63, head 2hj+1 on 64-127); per k-tile TWO concurrent
   K=64 matmuls (tile_position row groups 0 / 64) write the two halves of a
   2-bank PSUM pair -> score stream time halves vs zero-padded K=128.
 * exp on ACT per k-tile over the [128, 2x512] pair (diag tiles use a
   strided AP to skip fully-masked leading columns of both halves).
 * AV lags one k-tile behind exp so the PE never waits on ACT; causal mask
   via Pool multiply with a triangle (off the DVE).
 * qk/v projections of block b+1 and the output projection of block b-1 are
   generators, consumed 2 matmuls per attention iteration (the PE slack
   under the 1146ns exp pace), leftovers drained at pair/block ends.
 * normalize = PE ones-broadcast of the denominator row + DVE
   reciprocal_approx_fast + multiply, deferred one head-pair.
"""

import sys

for _p in ("/opt/trn_rl_repo", "/root/.axon_site/_ro/trn_rl_repo"):
    if _p not in sys.path:
        sys.path.append(_p)

from contextlib import ExitStack

import numpy as np
import ml_dtypes

import concourse.bass as bass
import concourse.tile as tile
from concourse import bacc, mybir
from concourse.bass_utils import run_bass_kernel_spmd
from concourse.masks import make_identity

F32 = mybir.dt.float32
F16 = mybir.dt.float16
BF16 = mybir.dt.bfloat16

B, L, D, H = 2, 2048, 1024, 16
DK = D // H  # 64
NCORES = 8
GH = 4  # heads per core
C = GH * DK  # 256: per-core slice of the qkv/head dim
QT_TILES = L // 512  # 4
DCH = D // 128  # 8


class Weave:
    """FIFO of generators; take(n) advances up to n emission steps."""

    def __init__(self, *gens):
        self.gens = list(gens)

    def push(self, gen):
        self.gens.append(gen)

    def take(self, n):
        while n > 0 and self.gens:
            try:
                next(self.gens[0])
                n -= 1
            except StopIteration:
                self.gens.pop(0)

    def drain(self):
        while self.gens:
            try:
                next(self.gens[0])
            except StopIteration:
                self.gens.pop(0)

    def finish_first(self):
        if self.gens:
            for _ in self.gens[0]:
                pass
            self.gens.pop(0)


def _build_program():
    nc = bacc.Bacc("TRN2", target_bir_lowering=False, debug=False, num_devices=NCORES)

    xt_d = nc.dram_tensor("xt", [128, DCH, L], BF16, kind="ExternalInput").ap()
    wq_d = nc.dram_tensor("wqt", [128, 2, DCH, 128], BF16, kind="ExternalInput").ap()
    wk_d = nc.dram_tensor("wkt", [128, 2, DCH, 128], BF16, kind="ExternalInput").ap()
    wv_d = nc.dram_tensor("wvt", [128, 2, DCH, 128], BF16, kind="ExternalInput").ap()
    wo_d = nc.dram_tensor("wot", [128, 2, D], BF16, kind="ExternalInput").ap()
    b6_d = nc.dram_tensor("b6", [128, 6], F32, kind="ExternalInput").ap()
    out_d = nc.dram_tensor("out", [L, D], BF16, kind="ExternalOutput").ap()

    with tile.TileContext(nc) as tc, ExitStack() as ctx:
        pool = ctx.enter_context(tc.tile_pool(name="persist", bufs=1))
        psum = ctx.enter_context(tc.tile_pool(name="psum", bufs=2, space="PSUM"))
        psum_o = ctx.enter_context(tc.tile_pool(name="psum_o", bufs=2, space="PSUM"))
        psum2 = ctx.enter_context(tc.tile_pool(name="psum2", bufs=2, space="PSUM"))
        cp = ctx.enter_context(tc.tile_pool(name="copies", bufs=3))
        yp = ctx.enter_context(tc.tile_pool(name="youts", bufs=2))

        XT = pool.tile([128, DCH, L], BF16)
        WT = {}
        for name in ("q", "k", "v"):
            WT[name] = pool.tile([128, 2, DCH, 128], BF16, name=f"W{name}T")
        WoT = pool.tile([128, 2, D], BF16)
        b6_sb = pool.tile([128, 6], F32)

        # ---- prologue DMAs.  The ~2.5 MB that gates block-0's projections
        # (wk/wq j=0 halves, x block 0, wv) is split evenly across the two
        # HWDGE rings in consumption order so the k/q/v matmul streams start
        # as chunks land; everything later-needed follows.
        nc.sync.dma_start(WT["k"][:, 0], wk_d[:, 0])
        for dci in range(0, 4):
            nc.sync.dma_start(XT[:, dci, 0:512], xt_d[:, dci, 0:512])
        nc.scalar.dma_start(WT["q"][:, 0], wq_d[:, 0])
        nc.scalar.dma_start(b6_sb[:], b6_d[:])
        for dci in range(4, 8):
            nc.scalar.dma_start(XT[:, dci, 0:512], xt_d[:, dci, 0:512])
        nc.sync.dma_start(WT["v"][:, 0], wv_d[:, 0])
        nc.sync.dma_start(WT["v"][:, 1], wv_d[:, 1])
        nc.scalar.dma_start(WT["k"][:, 1], wk_d[:, 1])
        nc.scalar.dma_start(WT["q"][:, 1], wq_d[:, 1])
        for dci in range(0, 4):
            nc.sync.dma_start(XT[:, dci, 512:1024], xt_d[:, dci, 512:1024])
        for dci in range(4, 8):
            nc.scalar.dma_start(XT[:, dci, 512:1024], xt_d[:, dci, 512:1024])
        nc.sync.dma_start(WoT[:], wo_d[:])
        for dci in range(DCH):
            nc.sync.dma_start(XT[:, dci, 1024:2048], xt_d[:, dci, 1024:2048])

        BIAS_COL = {"q": 0, "k": 2, "v": 4}

        ones_f32 = pool.tile([1, 128], F32)
        nc.gpsimd.memset(ones_f32[:], 1.0)
        ones_r = pool.tile([1, 128], mybir.dt.float32r)
        nc.vector.tensor_copy(ones_r[:], ones_f32[:])
        ident16 = pool.tile([128, 128], F16)
        make_identity(nc, ident16)
        tri_mask = pool.tile([128, 128], F16)
        nc.gpsimd.memset(tri_mask[:], 1.0)
        nc.gpsimd.affine_select(
            out=tri_mask[:],
            in_=tri_mask[:],
            pattern=[[1, 128]],
            compare_op=mybir.AluOpType.is_ge,
            fill=0.0,
            base=0,
            channel_multiplier=-1,
        )

        QTs = [pool.tile([128, 2, 512], F16, name=f"QT{g}") for g in range(4)]
        # k^T zero-padded per head to K=128 rows: the PE HAM clock gate only
        # un-throttles when matmuls stream all 128 partitions (64-row packed
        # score tiles measured to re-throttle the clock mid-kernel).
        KTzs = [pool.tile([128, GH, 512], F16, name=f"KTz{g}") for g in range(4)]
        Vp16 = [pool.tile([128, 4, GH, DK + 1], F16, name=f"Vp16_{g}") for g in range(4)]
        OTs = [pool.tile([128, 2, 512], BF16, name=f"OT{g}") for g in range(4)]

        for g in range(4):
            for h in range(GH):
                zp = 64 - 64 * (h % 2)
                nc.gpsimd.memset(KTzs[g][zp : zp + 64, h, :], 0.0)
        for g in range(4):
            nc.gpsimd.memset(Vp16[g][:, :, :, DK], 1.0)

        with nc.allow_low_precision(reason="bf16/f16 matmul inputs"):

            def kq_gen(blk, j):
                # k then q projection of one 128-col slice; yields per PE op.
                for name in ("k", "q"):
                    ps = psum.tile([128, 512], F32, tag="ps")
                    for dci in range(DCH):
                        nc.tensor.matmul(
                            ps[:],
                            lhsT=WT[name][:, j, dci, :],
                            rhs=XT[:, dci, blk * 512 : (blk + 1) * 512],
                            start=(dci == 0),
                            stop=(dci == DCH - 1),
                        )
                        yield
                    if name == "q":
                        nc.vector.tensor_tensor(
                            QTs[blk][:, j, :],
                            ps[:],
                            b6_sb[:, BIAS_COL["q"] + j, None].to_broadcast(
                                (128, 512)
                            ),
                            mybir.AluOpType.add,
                        )
                    else:
                        for half in range(2):
                            hp = 64 * half
                            nc.vector.tensor_tensor(
                                KTzs[blk][hp : hp + 64, 2 * j + half, :],
                                ps[hp : hp + 64, :],
                                b6_sb[
                                    hp : hp + 64, BIAS_COL["k"] + j, None
                                ].to_broadcast((64, 512)),
                                mybir.AluOpType.add,
                            )
                    yield

            def v_gen(blk):
                vt = cp.tile([128, 2, 512], F16, tag="vt", bufs=2)
                for ch in range(2):
                    ps = psum.tile([128, 512], F32, tag="ps")
                    for dci in range(DCH):
                        nc.tensor.matmul(
                            ps[:],
                            lhsT=WT["v"][:, ch, dci, :],
                            rhs=XT[:, dci, blk * 512 : (blk + 1) * 512],
                            start=(dci == 0),
                            stop=(dci == DCH - 1),
                        )
                        yield
                    nc.vector.tensor_tensor(
                        vt[:, ch, :],
                        ps[:],
                        b6_sb[:, BIAS_COL["v"] + ch, None].to_broadcast((128, 512)),
                        mybir.AluOpType.add,
                    )
                    yield
                for lsub in range(4):
                    pv = psum.tile([128, 256], F16, tag="ps")
                    for ch in range(2):
                        nc.tensor.matmul(
                            pv[:, ch * 128 : (ch + 1) * 128],
                            lhsT=vt[:, ch, lsub * 128 : (lsub + 1) * 128],
                            rhs=ident16[:],
                            is_transpose=True,
                            start=(ch == 0),
                            stop=(ch == 1),
                        )
                        yield
                    nc.vector.tensor_copy(
                        Vp16[blk][:, lsub, :, 0:DK],
                        pv[:].rearrange("p (h d) -> p h d", h=GH),
                    )
                    yield

            def chain(*gens):
                for g in gens:
                    yield from g

            def proj_gen(blk):
                # j=0 k/q first, then v, then j=1: attention on the block can
                # begin (heads 0/1) before the j=1 half exists.
                return chain(kq_gen(blk, 0), v_gen(blk), kq_gen(blk, 1))

            def normalize(h, qt, pso):
                hj, hp = h // 2, 64 * (h % 2)
                den_r = cp.tile([1, 512], mybir.dt.float32r, tag="den", bufs=2)
                nc.vector.tensor_copy(den_r[:], pso[64:65, :])
                psb = psum.tile([128, 512], F32, tag="ps")
                nc.tensor.matmul(
                    psb[:64], lhsT=ones_r[:, 0:64], rhs=den_r[:], start=True, stop=True
                )
                rb = cp.tile([64, 512], F32, tag="rb", bufs=2)
                nc.vector.reciprocal_approx_fast(rb[:], psb[:64])
                nc.vector.tensor_tensor(
                    OTs[qt][hp : hp + 64, hj, :],
                    pso[:64],
                    rb[:],
                    mybir.AluOpType.mult,
                )

            def normalize_pair(hj, qt, pso_e, pso_o):
                normalize(2 * hj, qt, pso_e)
                normalize(2 * hj + 1, qt, pso_o)

            def outproj_gen(qt512):
                for sub in range(4):
                    for e in range(2):
                        psy = psum.tile([128, 512], F32, tag="ps")
                        for cj in range(2):
                            nc.tensor.matmul(
                                psy[:],
                                lhsT=OTs[qt512][:, cj, sub * 128 : (sub + 1) * 128],
                                rhs=WoT[:, cj, e * 512 : (e + 1) * 512],
                                start=(cj == 0),
                                stop=(cj == 1),
                            )
                            yield
                        y_sb = yp.tile([128, 512], BF16, tag="y")
                        nc.vector.tensor_copy(y_sb[:], psy[:])
                        q0 = qt512 * 512 + sub * 128
                        nc.sync.dma_start(
                            out_d[q0 : q0 + 128, e * 512 : (e + 1) * 512], y_sb[:]
                        )
                        yield

            y3 = {}
            y8 = ctx.enter_context(tc.tile_pool(name="ytail", bufs=8))

            def outproj_half_gen(qt512, cj, subs=(0, 1, 2, 3)):
                # last-block outproj split by contraction half: cj=0 runs
                # during the final attention pair, cj=1 + accumulate + DMA is
                # all that trails the final normalize.
                for sub in subs:
                    for e in range(2):
                        idx = sub * 2 + e
                        psy = psum.tile([128, 512], F32, tag="ps")
                        nc.tensor.matmul(
                            psy[:],
                            lhsT=OTs[qt512][:, cj, sub * 128 : (sub + 1) * 128],
                            rhs=WoT[:, cj, e * 512 : (e + 1) * 512],
                            start=True,
                            stop=True,
                        )
                        yield
                        if cj == 0:
                            y_sb = y8.tile([128, 512], F32, tag="y8")
                            nc.vector.tensor_copy(y_sb[:], psy[:])
                            y3[idx] = y_sb
                        else:
                            y_sb = y3[idx]
                            yb = y8.tile([128, 512], BF16, tag="yb8")
                            nc.vector.tensor_tensor(
                                yb[:], y_sb[:], psy[:], mybir.AluOpType.add
                            )
                            q0 = qt512 * 512 + sub * 128
                            eng = nc.sync if idx % 2 == 0 else nc.scalar
                            eng.dma_start(
                                out_d[q0 : q0 + 128, e * 512 : (e + 1) * 512], yb[:]
                            )
                        yield

            pending = None

            def attn(qt, weave):
                nonlocal pending
                n_kt = 4 * qt + 4
                for hj in range(2):
                    if qt == 0 and hj == 1:
                        # block-0 only: finish j=1 projections before heads 2/3
                        weave.finish_first()
                    pso_e = psum_o.tile([128, 512], F32, tag="pso")
                    pso_o = psum_o.tile([128, 512], F32, tag="pso")
                    avq = []

                    def emit_av(kt, p_sb, d0):
                        g, ksub = kt // 4, kt % 4
                        for i, pso in ((0, pso_e), (1, pso_o)):
                            nc.tensor.matmul(
                                pso[:65, d0:],
                                lhsT=Vp16[g][:, ksub, 2 * hj + i, :],
                                rhs=p_sb[:, i * 512 + d0 : (i + 1) * 512],
                                start=(kt == 0),
                                stop=(kt == n_kt - 1),
                                skip_group_check=True,
                            )

                    for kt in range(n_kt):
                        g, ksub = kt // 4, kt % 4
                        pss = psum2.tile([128, 1024], F32, tag="ps2")
                        d0 = max(0, (kt - 4 * qt) * 128)
                        for i in range(2):  # the head pair, full-K=128 each
                            nc.tensor.matmul(
                                pss[:, i * 512 + d0 : (i + 1) * 512],
                                lhsT=KTzs[g][
                                    :, 2 * hj + i, ksub * 128 : (ksub + 1) * 128
                                ],
                                rhs=QTs[qt][:, hj, d0:],
                                start=True,
                                stop=True,
                            )
                        p_sb = cp.tile([128, 1024], F16, tag="p", bufs=5)
                        if d0:
                            nc.scalar.activation(
                                p_sb[:].rearrange("p (i q) -> p i q", i=2)[:, :, d0:],
                                pss[:].rearrange("p (i q) -> p i q", i=2)[:, :, d0:],
                                mybir.ActivationFunctionType.Exp,
                                scale=0.125,
                            )
                        else:
                            nc.scalar.activation(
                                p_sb[:],
                                pss[:],
                                mybir.ActivationFunctionType.Exp,
                                scale=0.125,
                            )
                        if kt >= 4 * qt:  # diagonal tile: causal mask, Pool
                            for i in range(2):
                                nc.gpsimd.tensor_tensor(
                                    p_sb[:, i * 512 + d0 : i * 512 + d0 + 128],
                                    p_sb[:, i * 512 + d0 : i * 512 + d0 + 128],
                                    tri_mask[:],
                                    mybir.AluOpType.mult,
                                )
                        avq.append((kt, p_sb, d0))
                        # AV lags two k-tiles behind exp: by emission time its
                        # exp AND Pool masks have both finished -> no PE wait
                        if len(avq) > 2:
                            emit_av(*avq.pop(0))
                        elif kt == 0 and pending is not None:
                            normalize_pair(*pending)
                            pending = None
                            if qt == QT_TILES - 1 and hj == 1:
                                weave.push(outproj_half_gen(qt, 0))
                        weave.take(2 if len(avq) <= 2 else 1)
                    for a in avq:
                        emit_av(*a)
                    pending = (hj, qt, pso_e, pso_o)

            # ==== block 0: j=0 k/q + v eagerly, then attention on heads 0/1
            # with the j=1 projections (then block 1's) woven in.
            for _ in chain(kq_gen(0, 0), v_gen(0)):
                pass
            for blk in range(QT_TILES):
                gens = []
                if blk == 0:
                    gens.append(kq_gen(0, 1))
                if blk > 0:
                    gens.append(outproj_gen(blk - 1))
                if blk < QT_TILES - 1:
                    gens.append(proj_gen(blk + 1))
                weave = Weave(*gens)
                attn(blk, weave)
                if blk < QT_TILES - 1:
                    weave.drain()
            # final normalize first, then drain leftovers: the PE chews the
            # remaining output-projection matmuls while the normalize's DVE
            # chain runs (keeps HAM warm into the tail)
            normalize_pair(*pending)
            weave.drain()
            for _ in outproj_half_gen(QT_TILES - 1, 1):
                pass

    nc.compile()
    return nc


_NC_CACHE = None


def _get_program():
    global _NC_CACHE
    if _NC_CACHE is None:
        _NC_CACHE = _build_program()
    return _NC_CACHE


def _run(in_maps, trace=False, **kw):
    nc = _get_program()
    return run_bass_kernel_spmd(nc, in_maps, list(range(NCORES)), trace=trace, **kw)


def _chunked_T(a, nch):
    """[R, Cc] -> [128, nch, Cc] with [p, i, c] = a[i*128+p, c], bf16."""
    r, c = a.shape
    assert r == nch * 128
    return np.ascontiguousarray(
        a.reshape(nch, 128, c).transpose(1, 0, 2)
    ).astype(ml_dtypes.bfloat16)


def _jmajor_T(a):
    """[D, C=256] -> [128, 2, DCH, 128] with [p, j, i, c] = a[i*128+p, j*128+c]."""
    d, c = a.shape
    assert d == DCH * 128 and c == 256
    return np.ascontiguousarray(
        a.reshape(DCH, 128, 2, 128).transpose(1, 2, 0, 3)
    ).astype(ml_dtypes.bfloat16)


def _make_in_maps(x, Wq, bq, Wk, bk, Wv, bv, Wo, bo):
    x = np.asarray(x, dtype=np.float32)
    Wq, Wk, Wv, Wo = (np.asarray(w, dtype=np.float32) for w in (Wq, Wk, Wv, Wo))
    bq, bk, bv = (np.asarray(b, dtype=np.float32) for b in (bq, bk, bv))
    in_maps = []
    xts = [_chunked_T(x[b].T, DCH) for b in range(B)]
    for core in range(NCORES):
        b, g = divmod(core, 4)
        s = slice(g * C, (g + 1) * C)
        b6 = np.stack(
            [
                bq[s][0:128], bq[s][128:256],
                bk[s][0:128], bk[s][128:256],
                bv[s][0:128], bv[s][128:256],
            ],
            axis=1,
        )
        in_maps.append(
            {
                "xt": xts[b],
                "wqt": _jmajor_T(Wq[s, :].T),
                "wkt": _jmajor_T(Wk[s, :].T),
                "wvt": _jmajor_T(Wv[s, :].T),
                "wot": _chunked_T(Wo[:, s].T, 2),
                "b6": np.ascontiguousarray(b6, dtype=np.float32),
            }
        )
    return in_maps


def kernel(x, Wq, bq, Wk, bk, Wv, bv, Wo, bo, _trace=False, _trace_out=None, _tmpdir=None):
    in_maps = _make_in_maps(x, Wq, bq, Wk, bk, Wv, bv, Wo, bo)
    res = _run(in_maps, trace=_trace, tmpdir=_tmpdir)
    if _trace_out is not None:
        _trace_out.append(res)
    bo = np.asarray(bo, dtype=np.float32)
    out = np.empty((B, L, D), dtype=np.float32)
    for b in range(B):
        acc = res.results[4 * b]["out"].astype(np.float32)
        for g in range(1, 4):
            acc = acc + res.results[4 * b + g]["out"].astype(np.float32)
        out[b] = acc + bo[None, :]
    return out


# revision 31
# speedup vs baseline: 1.1974x; 1.0174x over previous
"""Multi-head causal self-attention (B=2, L=2048, D=1024, H=16) on 8 TRN2
NeuronCores.  ~170 us HW exec (v1 baseline 210-214 us).

Sharding: core c handles batch b = c // 4 and head group g = c % 4 (4 heads,
i.e. a 256-wide slice of the QKV output dim and the matching 256 rows of
Wo^T).  Each core computes a full (L, D) partial of the output projection;
the host sums the 4 partials per batch (bf16 on the wire) and adds bo.

Structure:
 * Host pre-transposes + pre-casts x / W slices to bf16 in the exact on-chip
   layout (XT [128,8,2048], W*T [128,2,8,128] j-major, WoT [128,2,1024]) --
   no on-chip transposes/casts; the old 43us of Pool casts + PE/XBAR
   transposes and the staged f32 weight loads are gone entirely.
 * Startup: the ~2.5 MB gating block-0 projections is split across BOTH
   HWDGE rings (sync: wk_j0 + x chunks; scalar: wq_j0 + biases + x chunks +
   wv) in consumption order; the k-projection streams as chunks land.
 * Attention per (qt, head-pair hj, k-tile kt): the two heads' score
   matmuls (K=128, k^T zero-padded per head -- 64-row packed tiles measured
   to re-throttle the PE HAM clock mid-kernel) write a 2-bank PSUM pair;
   ONE exp per k-tile covers both heads (diag tiles use a strided AP to
   skip fully-masked leading cols of both halves AND trim the score
   matmuls themselves); causal mask via Pool multiply with a triangle;
   AV lags TWO k-tiles behind exp so its exp + masks are done by PE issue
   time (lag 3 measured worse).  Denominator via the ones-column of Vp.
 * qk/v projections of block b+1 and the output projection of block b-1
   are generators, woven 1-2 matmuls per attention iteration into the PE
   slack under the 1146ns exp pace; leftovers drain at block ends.  Block
   0 runs kq_j0+v eagerly and weaves its own j=1 half into heads 0/1.
 * normalize = PE ones-broadcast of the denominator row + DVE
   reciprocal_approx_fast + multiply, deferred one head-pair.  Last
   block's output projection split by contraction half (cj0 woven into
   the final pairs, cj1 + bf16 adds + dual-ring stores in the tail).

Measured NOT to work: 64-row tile_position-packed score matmuls (PE runs
at 1.2 GHz while fully busy -- HAM only counts full-128-partition
streams); fp8 q/k projections (rel err 2.6e-2 > 2e-2 gate, numpy sim);
fp8 p (exp overflows e4m3 range); a [2,128] float32r ones tile for a
fused 2-head normalize broadcast (NEFF compile failure); AV lag 3
(+32 us); splitting the first wk DMA into chunks (+2 us, delays x);
draining leftover weave after the final normalize (+1.4 us).
"""

import sys

for _p in ("/opt/trn_rl_repo", "/root/.axon_site/_ro/trn_rl_repo"):
    if _p not in sys.path:
        sys.path.append(_p)

from contextlib import ExitStack

import numpy as np
import ml_dtypes

import concourse.bass as bass
import concourse.tile as tile
from concourse import bacc, mybir
from concourse.bass_utils import run_bass_kernel_spmd
from concourse.masks import make_identity

F32 = mybir.dt.float32
F16 = mybir.dt.float16
BF16 = mybir.dt.bfloat16

B, L, D, H = 2, 2048, 1024, 16
DK = D // H  # 64
NCORES = 8
GH = 4  # heads per core
C = GH * DK  # 256: per-core slice of the qkv/head dim
QT_TILES = L // 512  # 4
DCH = D // 128  # 8


class Weave:
    """FIFO of generators; take(n) advances up to n emission steps."""

    def __init__(self, *gens):
        self.gens = list(gens)

    def push(self, gen):
        self.gens.append(gen)

    def take(self, n):
        while n > 0 and self.gens:
            try:
                next(self.gens[0])
                n -= 1
            except StopIteration:
                self.gens.pop(0)

    def drain(self):
        while self.gens:
            try:
                next(self.gens[0])
            except StopIteration:
                self.gens.pop(0)

    def finish_first(self):
        if self.gens:
            for _ in self.gens[0]:
                pass
            self.gens.pop(0)


def _build_program():
    nc = bacc.Bacc("TRN2", target_bir_lowering=False, debug=False, num_devices=NCORES)

    xt_d = nc.dram_tensor("xt", [128, DCH, L], BF16, kind="ExternalInput").ap()
    wq_d = nc.dram_tensor("wqt", [128, 2, DCH, 128], BF16, kind="ExternalInput").ap()
    wk_d = nc.dram_tensor("wkt", [128, 2, DCH, 128], BF16, kind="ExternalInput").ap()
    wv_d = nc.dram_tensor("wvt", [128, 2, DCH, 128], BF16, kind="ExternalInput").ap()
    wo_d = nc.dram_tensor("wot", [128, 2, D], BF16, kind="ExternalInput").ap()
    b6_d = nc.dram_tensor("b6", [128, 6], F32, kind="ExternalInput").ap()
    out_d = nc.dram_tensor("out", [L, D], BF16, kind="ExternalOutput").ap()

    with tile.TileContext(nc) as tc, ExitStack() as ctx:
        pool = ctx.enter_context(tc.tile_pool(name="persist", bufs=1))
        psum = ctx.enter_context(tc.tile_pool(name="psum", bufs=2, space="PSUM"))
        psum_o = ctx.enter_context(tc.tile_pool(name="psum_o", bufs=2, space="PSUM"))
        psum2 = ctx.enter_context(tc.tile_pool(name="psum2", bufs=2, space="PSUM"))
        cp = ctx.enter_context(tc.tile_pool(name="copies", bufs=3))
        yp = ctx.enter_context(tc.tile_pool(name="youts", bufs=2))

        XT = pool.tile([128, DCH, L], BF16)
        WT = {}
        for name in ("q", "k", "v"):
            WT[name] = pool.tile([128, 2, DCH, 128], BF16, name=f"W{name}T")
        WoT = pool.tile([128, 2, D], BF16)
        b6_sb = pool.tile([128, 6], F32)

        # ---- prologue DMAs.  The ~2.5 MB that gates block-0's projections
        # (wk/wq j=0 halves, x block 0, wv) is split evenly across the two
        # HWDGE rings in consumption order so the k/q/v matmul streams start
        # as chunks land; everything later-needed follows.
        nc.sync.dma_start(WT["k"][:, 0], wk_d[:, 0])
        for dci in range(0, 4):
            nc.sync.dma_start(XT[:, dci, 0:512], xt_d[:, dci, 0:512])
        nc.scalar.dma_start(WT["q"][:, 0], wq_d[:, 0])
        nc.scalar.dma_start(b6_sb[:], b6_d[:])
        for dci in range(4, 8):
            nc.scalar.dma_start(XT[:, dci, 0:512], xt_d[:, dci, 0:512])
        nc.sync.dma_start(WT["v"][:, 0], wv_d[:, 0])
        nc.sync.dma_start(WT["v"][:, 1], wv_d[:, 1])
        nc.scalar.dma_start(WT["k"][:, 1], wk_d[:, 1])
        nc.scalar.dma_start(WT["q"][:, 1], wq_d[:, 1])
        for dci in range(0, 4):
            nc.sync.dma_start(XT[:, dci, 512:1024], xt_d[:, dci, 512:1024])
        for dci in range(4, 8):
            nc.scalar.dma_start(XT[:, dci, 512:1024], xt_d[:, dci, 512:1024])
        nc.sync.dma_start(WoT[:], wo_d[:])
        for dci in range(DCH):
            nc.sync.dma_start(XT[:, dci, 1024:2048], xt_d[:, dci, 1024:2048])

        BIAS_COL = {"q": 0, "k": 2, "v": 4}

        ones_f32 = pool.tile([1, 128], F32)
        nc.gpsimd.memset(ones_f32[:], 1.0)
        ones_r = pool.tile([1, 128], mybir.dt.float32r)
        nc.vector.tensor_copy(ones_r[:], ones_f32[:])
        ident16 = pool.tile([128, 128], F16)
        make_identity(nc, ident16)
        tri_mask = pool.tile([128, 128], F16)
        nc.gpsimd.memset(tri_mask[:], 1.0)
        nc.gpsimd.affine_select(
            out=tri_mask[:],
            in_=tri_mask[:],
            pattern=[[1, 128]],
            compare_op=mybir.AluOpType.is_ge,
            fill=0.0,
            base=0,
            channel_multiplier=-1,
        )

        QTs = [pool.tile([128, 2, 512], F16, name=f"QT{g}") for g in range(4)]
        # k^T zero-padded per head to K=128 rows: the PE HAM clock gate only
        # un-throttles when matmuls stream all 128 partitions (64-row packed
        # score tiles measured to re-throttle the clock mid-kernel).
        KTzs = [pool.tile([128, GH, 512], F16, name=f"KTz{g}") for g in range(4)]
        Vp16 = [pool.tile([128, 4, GH, DK + 1], F16, name=f"Vp16_{g}") for g in range(4)]
        OTs = [pool.tile([128, 2, 512], BF16, name=f"OT{g}") for g in range(4)]

        for g in range(4):
            for h in range(GH):
                zp = 64 - 64 * (h % 2)
                nc.gpsimd.memset(KTzs[g][zp : zp + 64, h, :], 0.0)
        for g in range(4):
            nc.gpsimd.memset(Vp16[g][:, :, :, DK], 1.0)

        with nc.allow_low_precision(reason="bf16/f16 matmul inputs"):

            def kq_gen(blk, j):
                # k then q projection of one 128-col slice; yields per PE op.
                for name in ("k", "q"):
                    ps = psum.tile([128, 512], F32, tag="ps")
                    for dci in range(DCH):
                        nc.tensor.matmul(
                            ps[:],
                            lhsT=WT[name][:, j, dci, :],
                            rhs=XT[:, dci, blk * 512 : (blk + 1) * 512],
                            start=(dci == 0),
                            stop=(dci == DCH - 1),
                        )
                        yield
                    if name == "q":
                        nc.vector.tensor_tensor(
                            QTs[blk][:, j, :],
                            ps[:],
                            b6_sb[:, BIAS_COL["q"] + j, None].to_broadcast(
                                (128, 512)
                            ),
                            mybir.AluOpType.add,
                        )
                    else:
                        for half in range(2):
                            hp = 64 * half
                            nc.vector.tensor_tensor(
                                KTzs[blk][hp : hp + 64, 2 * j + half, :],
                                ps[hp : hp + 64, :],
                                b6_sb[
                                    hp : hp + 64, BIAS_COL["k"] + j, None
                                ].to_broadcast((64, 512)),
                                mybir.AluOpType.add,
                            )
                    yield

            def v_gen(blk):
                vt = cp.tile([128, 2, 512], F16, tag="vt", bufs=2)
                for ch in range(2):
                    ps = psum.tile([128, 512], F32, tag="ps")
                    for dci in range(DCH):
                        nc.tensor.matmul(
                            ps[:],
                            lhsT=WT["v"][:, ch, dci, :],
                            rhs=XT[:, dci, blk * 512 : (blk + 1) * 512],
                            start=(dci == 0),
                            stop=(dci == DCH - 1),
                        )
                        yield
                    nc.vector.tensor_tensor(
                        vt[:, ch, :],
                        ps[:],
                        b6_sb[:, BIAS_COL["v"] + ch, None].to_broadcast((128, 512)),
                        mybir.AluOpType.add,
                    )
                    yield
                for lsub in range(4):
                    pv = psum.tile([128, 256], F16, tag="ps")
                    for ch in range(2):
                        nc.tensor.matmul(
                            pv[:, ch * 128 : (ch + 1) * 128],
                            lhsT=vt[:, ch, lsub * 128 : (lsub + 1) * 128],
                            rhs=ident16[:],
                            is_transpose=True,
                            start=(ch == 0),
                            stop=(ch == 1),
                        )
                        yield
                    nc.vector.tensor_copy(
                        Vp16[blk][:, lsub, :, 0:DK],
                        pv[:].rearrange("p (h d) -> p h d", h=GH),
                    )
                    yield

            def chain(*gens):
                for g in gens:
                    yield from g

            def proj_gen(blk):
                # j=0 k/q first, then v, then j=1: attention on the block can
                # begin (heads 0/1) before the j=1 half exists.
                return chain(kq_gen(blk, 0), v_gen(blk), kq_gen(blk, 1))

            def normalize(h, qt, pso):
                hj, hp = h // 2, 64 * (h % 2)
                den_r = cp.tile([1, 512], mybir.dt.float32r, tag="den", bufs=2)
                nc.vector.tensor_copy(den_r[:], pso[64:65, :])
                psb = psum.tile([128, 512], F32, tag="ps")
                nc.tensor.matmul(
                    psb[:64], lhsT=ones_r[:, 0:64], rhs=den_r[:], start=True, stop=True
                )
                rb = cp.tile([64, 512], F32, tag="rb", bufs=2)
                nc.vector.reciprocal_approx_fast(rb[:], psb[:64])
                nc.vector.tensor_tensor(
                    OTs[qt][hp : hp + 64, hj, :],
                    pso[:64],
                    rb[:],
                    mybir.AluOpType.mult,
                )

            def normalize_pair(hj, qt, pso_e, pso_o):
                normalize(2 * hj, qt, pso_e)
                normalize(2 * hj + 1, qt, pso_o)

            def outproj_gen(qt512):
                for sub in range(4):
                    for e in range(2):
                        psy = psum.tile([128, 512], F32, tag="ps")
                        for cj in range(2):
                            nc.tensor.matmul(
                                psy[:],
                                lhsT=OTs[qt512][:, cj, sub * 128 : (sub + 1) * 128],
                                rhs=WoT[:, cj, e * 512 : (e + 1) * 512],
                                start=(cj == 0),
                                stop=(cj == 1),
                            )
                            yield
                        y_sb = yp.tile([128, 512], BF16, tag="y")
                        nc.vector.tensor_copy(y_sb[:], psy[:])
                        q0 = qt512 * 512 + sub * 128
                        nc.sync.dma_start(
                            out_d[q0 : q0 + 128, e * 512 : (e + 1) * 512], y_sb[:]
                        )
                        yield

            y3 = {}
            y8 = ctx.enter_context(tc.tile_pool(name="ytail", bufs=8))

            def outproj_half_gen(qt512, cj, subs=(0, 1, 2, 3)):
                # last-block outproj split by contraction half: cj=0 runs
                # during the final attention pair, cj=1 + accumulate + DMA is
                # all that trails the final normalize.
                for sub in subs:
                    for e in range(2):
                        idx = sub * 2 + e
                        psy = psum.tile([128, 512], F32, tag="ps")
                        nc.tensor.matmul(
                            psy[:],
                            lhsT=OTs[qt512][:, cj, sub * 128 : (sub + 1) * 128],
                            rhs=WoT[:, cj, e * 512 : (e + 1) * 512],
                            start=True,
                            stop=True,
                        )
                        yield
                        if cj == 0:
                            y_sb = y8.tile([128, 512], F32, tag="y8")
                            nc.vector.tensor_copy(y_sb[:], psy[:])
                            y3[idx] = y_sb
                        else:
                            y_sb = y3[idx]
                            yb = y8.tile([128, 512], BF16, tag="yb8")
                            nc.vector.tensor_tensor(
                                yb[:], y_sb[:], psy[:], mybir.AluOpType.add
                            )
                            q0 = qt512 * 512 + sub * 128
                            eng = nc.sync if idx % 2 == 0 else nc.scalar
                            eng.dma_start(
                                out_d[q0 : q0 + 128, e * 512 : (e + 1) * 512], yb[:]
                            )
                        yield

            pending = None

            def attn(qt, weave):
                nonlocal pending
                n_kt = 4 * qt + 4
                for hj in range(2):
                    if qt == 0 and hj == 1:
                        # block-0 only: finish j=1 projections before heads 2/3
                        weave.finish_first()
                    pso_e = psum_o.tile([128, 512], F32, tag="pso")
                    pso_o = psum_o.tile([128, 512], F32, tag="pso")
                    avq = []

                    def emit_av(kt, p_sb, d0):
                        g, ksub = kt // 4, kt % 4
                        for i, pso in ((0, pso_e), (1, pso_o)):
                            nc.tensor.matmul(
                                pso[:65, d0:],
                                lhsT=Vp16[g][:, ksub, 2 * hj + i, :],
                                rhs=p_sb[:, i * 512 + d0 : (i + 1) * 512],
                                start=(kt == 0),
                                stop=(kt == n_kt - 1),
                                skip_group_check=True,
                            )

                    for kt in range(n_kt):
                        g, ksub = kt // 4, kt % 4
                        pss = psum2.tile([128, 1024], F32, tag="ps2")
                        d0 = max(0, (kt - 4 * qt) * 128)
                        for i in range(2):  # the head pair, full-K=128 each
                            nc.tensor.matmul(
                                pss[:, i * 512 + d0 : (i + 1) * 512],
                                lhsT=KTzs[g][
                                    :, 2 * hj + i, ksub * 128 : (ksub + 1) * 128
                                ],
                                rhs=QTs[qt][:, hj, d0:],
                                start=True,
                                stop=True,
                            )
                        p_sb = cp.tile([128, 1024], F16, tag="p", bufs=5)
                        if d0:
                            nc.scalar.activation(
                                p_sb[:].rearrange("p (i q) -> p i q", i=2)[:, :, d0:],
                                pss[:].rearrange("p (i q) -> p i q", i=2)[:, :, d0:],
                                mybir.ActivationFunctionType.Exp,
                                scale=0.125,
                            )
                        else:
                            nc.scalar.activation(
                                p_sb[:],
                                pss[:],
                                mybir.ActivationFunctionType.Exp,
                                scale=0.125,
                            )
                        if kt >= 4 * qt:  # diagonal tile: causal mask, Pool
                            for i in range(2):
                                nc.gpsimd.tensor_tensor(
                                    p_sb[:, i * 512 + d0 : i * 512 + d0 + 128],
                                    p_sb[:, i * 512 + d0 : i * 512 + d0 + 128],
                                    tri_mask[:],
                                    mybir.AluOpType.mult,
                                )
                        avq.append((kt, p_sb, d0))
                        # AV lags two k-tiles behind exp: by emission time its
                        # exp AND Pool masks have both finished -> no PE wait
                        if len(avq) > 2:
                            emit_av(*avq.pop(0))
                        elif kt == 0 and pending is not None:
                            normalize_pair(*pending)
                            pending = None
                            if qt == QT_TILES - 1 and hj == 1:
                                weave.push(outproj_half_gen(qt, 0))
                        weave.take(2 if len(avq) <= 2 else 1)
                    for a in avq:
                        emit_av(*a)
                    pending = (hj, qt, pso_e, pso_o)

            # ==== block 0: j=0 k/q + v eagerly, then attention on heads 0/1
            # with the j=1 projections (then block 1's) woven in.
            for _ in chain(kq_gen(0, 0), v_gen(0)):
                pass
            for blk in range(QT_TILES):
                gens = []
                if blk == 0:
                    gens.append(kq_gen(0, 1))
                if blk > 0:
                    gens.append(outproj_gen(blk - 1))
                if blk < QT_TILES - 1:
                    gens.append(proj_gen(blk + 1))
                weave = Weave(*gens)
                attn(blk, weave)
                weave.drain()
            normalize_pair(*pending)
            for _ in outproj_half_gen(QT_TILES - 1, 1):
                pass

    nc.compile()
    return nc


_NC_CACHE = None


def _get_program():
    global _NC_CACHE
    if _NC_CACHE is None:
        _NC_CACHE = _build_program()
    return _NC_CACHE


def _run(in_maps, trace=False, **kw):
    nc = _get_program()
    return run_bass_kernel_spmd(nc, in_maps, list(range(NCORES)), trace=trace, **kw)


def _chunked_T(a, nch):
    """[R, Cc] -> [128, nch, Cc] with [p, i, c] = a[i*128+p, c], bf16."""
    r, c = a.shape
    assert r == nch * 128
    return np.ascontiguousarray(
        a.reshape(nch, 128, c).transpose(1, 0, 2)
    ).astype(ml_dtypes.bfloat16)


def _jmajor_T(a):
    """[D, C=256] -> [128, 2, DCH, 128] with [p, j, i, c] = a[i*128+p, j*128+c]."""
    d, c = a.shape
    assert d == DCH * 128 and c == 256
    return np.ascontiguousarray(
        a.reshape(DCH, 128, 2, 128).transpose(1, 2, 0, 3)
    ).astype(ml_dtypes.bfloat16)


def _make_in_maps(x, Wq, bq, Wk, bk, Wv, bv, Wo, bo):
    x = np.asarray(x, dtype=np.float32)
    Wq, Wk, Wv, Wo = (np.asarray(w, dtype=np.float32) for w in (Wq, Wk, Wv, Wo))
    bq, bk, bv = (np.asarray(b, dtype=np.float32) for b in (bq, bk, bv))
    in_maps = []
    xts = [_chunked_T(x[b].T, DCH) for b in range(B)]
    for core in range(NCORES):
        b, g = divmod(core, 4)
        s = slice(g * C, (g + 1) * C)
        b6 = np.stack(
            [
                bq[s][0:128], bq[s][128:256],
                bk[s][0:128], bk[s][128:256],
                bv[s][0:128], bv[s][128:256],
            ],
            axis=1,
        )
        in_maps.append(
            {
                "xt": xts[b],
                "wqt": _jmajor_T(Wq[s, :].T),
                "wkt": _jmajor_T(Wk[s, :].T),
                "wvt": _jmajor_T(Wv[s, :].T),
                "wot": _chunked_T(Wo[:, s].T, 2),
                "b6": np.ascontiguousarray(b6, dtype=np.float32),
            }
        )
    return in_maps


def kernel(x, Wq, bq, Wk, bk, Wv, bv, Wo, bo, _trace=False, _trace_out=None, _tmpdir=None):
    in_maps = _make_in_maps(x, Wq, bq, Wk, bk, Wv, bv, Wo, bo)
    res = _run(in_maps, trace=_trace, tmpdir=_tmpdir)
    if _trace_out is not None:
        _trace_out.append(res)
    bo = np.asarray(bo, dtype=np.float32)
    out = np.empty((B, L, D), dtype=np.float32)
    for b in range(B):
        acc = res.results[4 * b]["out"].astype(np.float32)
        for g in range(1, 4):
            acc = acc + res.results[4 * b + g]["out"].astype(np.float32)
        out[b] = acc + bo[None, :]
    return out


# revision 33
# speedup vs baseline: 1.2065x; 1.0076x over previous
"""Multi-head causal self-attention (B=2, L=2048, D=1024, H=16) on 8 TRN2
NeuronCores.  ~170 us HW exec (v1 baseline 210-214 us).

Sharding: core c handles batch b = c // 4 and head group g = c % 4 (4 heads,
i.e. a 256-wide slice of the QKV output dim and the matching 256 rows of
Wo^T).  Each core computes a full (L, D) partial of the output projection;
the host sums the 4 partials per batch (bf16 on the wire) and adds bo.

Structure:
 * Host pre-transposes + pre-casts x / W slices to bf16 in the exact on-chip
   layout (XT [128,8,2048], W*T [128,2,8,128] j-major, WoT [128,2,1024]) --
   no on-chip transposes/casts; the old 43us of Pool casts + PE/XBAR
   transposes and the staged f32 weight loads are gone entirely.
 * Startup: the ~2.5 MB gating block-0 projections is split across BOTH
   HWDGE rings (sync: wk_j0 + x chunks; scalar: wq_j0 + biases + x chunks +
   wv) in consumption order; the k-projection streams as chunks land.
 * Attention per (qt, head-pair hj, k-tile kt): the two heads' score
   matmuls (K=128, k^T zero-padded per head -- 64-row packed tiles measured
   to re-throttle the PE HAM clock mid-kernel) write a 2-bank PSUM pair;
   ONE exp per k-tile covers both heads (diag tiles use a strided AP to
   skip fully-masked leading cols of both halves AND trim the score
   matmuls themselves); causal mask via Pool multiply with a triangle;
   AV lags TWO k-tiles behind exp so its exp + masks are done by PE issue
   time (lag 3 measured worse).  Denominator via the ones-column of Vp.
 * qk/v projections of block b+1 and the output projection of block b-1
   are generators, woven 1-2 matmuls per attention iteration into the PE
   slack under the 1146ns exp pace; leftovers drain at block ends.  Block
   0 runs kq_j0+v eagerly and weaves its own j=1 half into heads 0/1.
 * normalize = PE ones-broadcast of the denominator row + DVE
   reciprocal_approx_fast + multiply, deferred one head-pair.  Last
   block's output projection split by contraction half (cj0 woven into
   the final pairs, cj1 + bf16 adds + dual-ring stores in the tail).

Measured NOT to work: 64-row tile_position-packed score matmuls (PE runs
at 1.2 GHz while fully busy -- HAM only counts full-128-partition
streams); fp8 q/k projections (rel err 2.6e-2 > 2e-2 gate, numpy sim);
fp8 p (exp overflows e4m3 range); a [2,128] float32r ones tile for a
fused 2-head normalize broadcast (NEFF compile failure); AV lag 3
(+32 us); splitting the first wk DMA into chunks (+2 us, delays x);
draining leftover weave after the final normalize (+1.4 us).
"""

import sys

for _p in ("/opt/trn_rl_repo", "/root/.axon_site/_ro/trn_rl_repo"):
    if _p not in sys.path:
        sys.path.append(_p)

from contextlib import ExitStack

import numpy as np
import ml_dtypes

import concourse.bass as bass
import concourse.tile as tile
from concourse import bacc, mybir
from concourse.bass_utils import run_bass_kernel_spmd
from concourse.masks import make_identity

F32 = mybir.dt.float32
F16 = mybir.dt.float16
BF16 = mybir.dt.bfloat16

B, L, D, H = 2, 2048, 1024, 16
DK = D // H  # 64
NCORES = 8
GH = 4  # heads per core
C = GH * DK  # 256: per-core slice of the qkv/head dim
QT_TILES = L // 512  # 4
DCH = D // 128  # 8


class Weave:
    """FIFO of generators; take(n) advances up to n emission steps."""

    def __init__(self, *gens):
        self.gens = list(gens)

    def push(self, gen):
        self.gens.append(gen)

    def take(self, n):
        while n > 0 and self.gens:
            try:
                next(self.gens[0])
                n -= 1
            except StopIteration:
                self.gens.pop(0)

    def drain(self):
        while self.gens:
            try:
                next(self.gens[0])
            except StopIteration:
                self.gens.pop(0)

    def finish_first(self):
        if self.gens:
            for _ in self.gens[0]:
                pass
            self.gens.pop(0)


def _build_program():
    nc = bacc.Bacc("TRN2", target_bir_lowering=False, debug=False, num_devices=NCORES)

    xt_d = nc.dram_tensor("xt", [128, DCH, L], BF16, kind="ExternalInput").ap()
    wq_d = nc.dram_tensor("wqt", [128, 2, DCH, 128], BF16, kind="ExternalInput").ap()
    wk_d = nc.dram_tensor("wkt", [128, 2, DCH, 128], BF16, kind="ExternalInput").ap()
    wv_d = nc.dram_tensor("wvt", [128, 2, DCH, 128], BF16, kind="ExternalInput").ap()
    wo_d = nc.dram_tensor("wot", [128, 2, D], BF16, kind="ExternalInput").ap()
    b6_d = nc.dram_tensor("b6", [128, 6], F32, kind="ExternalInput").ap()
    out_d = nc.dram_tensor("out", [L, D], BF16, kind="ExternalOutput").ap()

    with tile.TileContext(nc) as tc, ExitStack() as ctx:
        pool = ctx.enter_context(tc.tile_pool(name="persist", bufs=1))
        psum = ctx.enter_context(tc.tile_pool(name="psum", bufs=2, space="PSUM"))
        psum_o = ctx.enter_context(tc.tile_pool(name="psum_o", bufs=2, space="PSUM"))
        psum2 = ctx.enter_context(tc.tile_pool(name="psum2", bufs=2, space="PSUM"))
        cp = ctx.enter_context(tc.tile_pool(name="copies", bufs=3))
        yp = ctx.enter_context(tc.tile_pool(name="youts", bufs=2))

        XT = pool.tile([128, DCH, L], BF16)
        WT = {}
        for name in ("q", "k", "v"):
            WT[name] = pool.tile([128, 2, DCH, 128], BF16, name=f"W{name}T")
        WoT = pool.tile([128, 2, D], BF16)
        b6_sb = pool.tile([128, 6], F32)

        # ---- prologue DMAs.  The ~2.5 MB that gates block-0's projections
        # (wk/wq j=0 halves, x block 0, wv) is split evenly across the two
        # HWDGE rings in consumption order so the k/q/v matmul streams start
        # as chunks land; everything later-needed follows.
        nc.sync.dma_start(WT["k"][:, 0], wk_d[:, 0])
        for dci in range(0, 4):
            nc.sync.dma_start(XT[:, dci, 0:512], xt_d[:, dci, 0:512])
        nc.scalar.dma_start(WT["q"][:, 0], wq_d[:, 0])
        nc.scalar.dma_start(b6_sb[:], b6_d[:])
        for dci in range(4, 8):
            nc.scalar.dma_start(XT[:, dci, 0:512], xt_d[:, dci, 0:512])
        nc.sync.dma_start(WT["v"][:, 0], wv_d[:, 0])
        nc.sync.dma_start(WT["v"][:, 1], wv_d[:, 1])
        nc.scalar.dma_start(WT["k"][:, 1], wk_d[:, 1])
        nc.scalar.dma_start(WT["q"][:, 1], wq_d[:, 1])
        for dci in range(0, 4):
            nc.sync.dma_start(XT[:, dci, 512:1024], xt_d[:, dci, 512:1024])
        for dci in range(4, 8):
            nc.scalar.dma_start(XT[:, dci, 512:1024], xt_d[:, dci, 512:1024])
        nc.sync.dma_start(WoT[:], wo_d[:])
        for dci in range(DCH):
            nc.sync.dma_start(XT[:, dci, 1024:2048], xt_d[:, dci, 1024:2048])

        BIAS_COL = {"q": 0, "k": 2, "v": 4}

        ones_f32 = pool.tile([1, 128], F32)
        nc.gpsimd.memset(ones_f32[:], 1.0)
        ones_r = pool.tile([1, 128], mybir.dt.float32r)
        nc.vector.tensor_copy(ones_r[:], ones_f32[:])
        ident16 = pool.tile([128, 128], F16)
        make_identity(nc, ident16)
        tri_mask = pool.tile([128, 128], F16)
        nc.gpsimd.memset(tri_mask[:], 1.0)
        nc.gpsimd.affine_select(
            out=tri_mask[:],
            in_=tri_mask[:],
            pattern=[[1, 128]],
            compare_op=mybir.AluOpType.is_ge,
            fill=0.0,
            base=0,
            channel_multiplier=-1,
        )

        QTs = [pool.tile([128, 2, 512], F16, name=f"QT{g}") for g in range(4)]
        # k^T zero-padded per head to K=128 rows: the PE HAM clock gate only
        # un-throttles when matmuls stream all 128 partitions (64-row packed
        # score tiles measured to re-throttle the clock mid-kernel).
        KTzs = [pool.tile([128, GH, 512], F16, name=f"KTz{g}") for g in range(4)]
        Vp16 = [pool.tile([128, 4, GH, DK + 1], F16, name=f"Vp16_{g}") for g in range(4)]
        OTs = [pool.tile([128, 2, 512], BF16, name=f"OT{g}") for g in range(4)]

        for g in range(4):
            for h in range(GH):
                zp = 64 - 64 * (h % 2)
                nc.gpsimd.memset(KTzs[g][zp : zp + 64, h, :], 0.0)
        for g in range(4):
            nc.gpsimd.memset(Vp16[g][:, :, :, DK], 1.0)

        with nc.allow_low_precision(reason="bf16/f16 matmul inputs"):

            def kq_gen(blk, j):
                # k then q projection of one 128-col slice; yields per PE op.
                for name in ("k", "q"):
                    ps = psum.tile([128, 512], F32, tag="ps")
                    for dci in range(DCH):
                        nc.tensor.matmul(
                            ps[:],
                            lhsT=WT[name][:, j, dci, :],
                            rhs=XT[:, dci, blk * 512 : (blk + 1) * 512],
                            start=(dci == 0),
                            stop=(dci == DCH - 1),
                        )
                        yield
                    if name == "q":
                        nc.vector.tensor_tensor(
                            QTs[blk][:, j, :],
                            ps[:],
                            b6_sb[:, BIAS_COL["q"] + j, None].to_broadcast(
                                (128, 512)
                            ),
                            mybir.AluOpType.add,
                        )
                    else:
                        for half in range(2):
                            hp = 64 * half
                            nc.vector.tensor_tensor(
                                KTzs[blk][hp : hp + 64, 2 * j + half, :],
                                ps[hp : hp + 64, :],
                                b6_sb[
                                    hp : hp + 64, BIAS_COL["k"] + j, None
                                ].to_broadcast((64, 512)),
                                mybir.AluOpType.add,
                            )
                    yield

            def v_gen(blk):
                vt = cp.tile([128, 2, 512], F16, tag="vt", bufs=2)
                for ch in range(2):
                    ps = psum.tile([128, 512], F32, tag="ps")
                    for dci in range(DCH):
                        nc.tensor.matmul(
                            ps[:],
                            lhsT=WT["v"][:, ch, dci, :],
                            rhs=XT[:, dci, blk * 512 : (blk + 1) * 512],
                            start=(dci == 0),
                            stop=(dci == DCH - 1),
                        )
                        yield
                    nc.vector.tensor_tensor(
                        vt[:, ch, :],
                        ps[:],
                        b6_sb[:, BIAS_COL["v"] + ch, None].to_broadcast((128, 512)),
                        mybir.AluOpType.add,
                    )
                    yield
                for lsub in range(4):
                    pv = psum.tile([128, 256], F16, tag="ps")
                    for ch in range(2):
                        nc.tensor.matmul(
                            pv[:, ch * 128 : (ch + 1) * 128],
                            lhsT=vt[:, ch, lsub * 128 : (lsub + 1) * 128],
                            rhs=ident16[:],
                            is_transpose=True,
                            start=(ch == 0),
                            stop=(ch == 1),
                        )
                        yield
                    nc.vector.tensor_copy(
                        Vp16[blk][:, lsub, :, 0:DK],
                        pv[:].rearrange("p (h d) -> p h d", h=GH),
                    )
                    yield

            def chain(*gens):
                for g in gens:
                    yield from g

            def proj_gen(blk):
                # j=0 k/q first, then v, then j=1: attention on the block can
                # begin (heads 0/1) before the j=1 half exists.
                return chain(kq_gen(blk, 0), v_gen(blk), kq_gen(blk, 1))

            def normalize(h, qt, pso):
                hj, hp = h // 2, 64 * (h % 2)
                den_r = cp.tile([1, 512], mybir.dt.float32r, tag="den", bufs=2)
                nc.vector.tensor_copy(den_r[:], pso[64:65, :])
                psb = psum.tile([128, 512], F32, tag="ps")
                nc.tensor.matmul(
                    psb[:64], lhsT=ones_r[:, 0:64], rhs=den_r[:], start=True, stop=True
                )
                rb = cp.tile([64, 512], F32, tag="rb", bufs=2)
                nc.vector.reciprocal_approx_fast(rb[:], psb[:64])
                nc.vector.tensor_tensor(
                    OTs[qt][hp : hp + 64, hj, :],
                    pso[:64],
                    rb[:],
                    mybir.AluOpType.mult,
                )

            def normalize_pair(hj, qt, pso_e, pso_o):
                normalize(2 * hj, qt, pso_e)
                normalize(2 * hj + 1, qt, pso_o)

            def outproj_gen(qt512):
                for sub in range(4):
                    for e in range(2):
                        psy = psum.tile([128, 512], F32, tag="ps")
                        for cj in range(2):
                            nc.tensor.matmul(
                                psy[:],
                                lhsT=OTs[qt512][:, cj, sub * 128 : (sub + 1) * 128],
                                rhs=WoT[:, cj, e * 512 : (e + 1) * 512],
                                start=(cj == 0),
                                stop=(cj == 1),
                            )
                            yield
                        y_sb = yp.tile([128, 512], BF16, tag="y")
                        nc.vector.tensor_copy(y_sb[:], psy[:])
                        q0 = qt512 * 512 + sub * 128
                        nc.sync.dma_start(
                            out_d[q0 : q0 + 128, e * 512 : (e + 1) * 512], y_sb[:]
                        )
                        yield

            y3 = {}
            y8 = ctx.enter_context(tc.tile_pool(name="ytail", bufs=8))

            def outproj_half_gen(qt512, cj, subs=(0, 1, 2, 3)):
                # last-block outproj split by contraction half: cj=0 runs
                # during the final attention pair, cj=1 + accumulate + DMA is
                # all that trails the final normalize.
                for sub in subs:
                    for e in range(2):
                        idx = sub * 2 + e
                        psy = psum.tile([128, 512], F32, tag="ps")
                        nc.tensor.matmul(
                            psy[:],
                            lhsT=OTs[qt512][:, cj, sub * 128 : (sub + 1) * 128],
                            rhs=WoT[:, cj, e * 512 : (e + 1) * 512],
                            start=True,
                            stop=True,
                        )
                        yield
                        if cj == 0:
                            y_sb = y8.tile([128, 512], F32, tag="y8")
                            nc.vector.tensor_copy(y_sb[:], psy[:])
                            y3[idx] = y_sb
                        else:
                            y_sb = y3[idx]
                            yb = y8.tile([128, 512], BF16, tag="yb8")
                            nc.vector.tensor_tensor(
                                yb[:], y_sb[:], psy[:], mybir.AluOpType.add
                            )
                            q0 = qt512 * 512 + sub * 128
                            eng = nc.sync if idx % 2 == 0 else nc.scalar
                            eng.dma_start(
                                out_d[q0 : q0 + 128, e * 512 : (e + 1) * 512], yb[:]
                            )
                        yield

            pending = None

            def attn(qt, weave):
                nonlocal pending
                n_kt = 4 * qt + 4
                for hj in range(2):
                    if qt == 0 and hj == 1:
                        # block-0 only: finish j=1 projections before heads 2/3
                        weave.finish_first()
                    pso_e = psum_o.tile([128, 512], F32, tag="pso")
                    pso_o = psum_o.tile([128, 512], F32, tag="pso")
                    avq = []

                    def emit_av(kt, p_sb, d0):
                        g, ksub = kt // 4, kt % 4
                        for i, pso in ((0, pso_e), (1, pso_o)):
                            nc.tensor.matmul(
                                pso[:65, d0:],
                                lhsT=Vp16[g][:, ksub, 2 * hj + i, :],
                                rhs=p_sb[:, i * 512 + d0 : (i + 1) * 512],
                                start=(kt == 0),
                                stop=(kt == n_kt - 1),
                                skip_group_check=True,
                            )

                    for kt in range(n_kt):
                        g, ksub = kt // 4, kt % 4
                        pss = psum2.tile([128, 1024], F32, tag="ps2")
                        d0 = max(0, (kt - 4 * qt) * 128)
                        for i in range(2):  # the head pair, full-K=128 each
                            nc.tensor.matmul(
                                pss[:, i * 512 + d0 : (i + 1) * 512],
                                lhsT=KTzs[g][
                                    :, 2 * hj + i, ksub * 128 : (ksub + 1) * 128
                                ],
                                rhs=QTs[qt][:, hj, d0:],
                                start=True,
                                stop=True,
                            )
                        p_sb = cp.tile([128, 1024], F16, tag="p", bufs=5)
                        if d0:
                            nc.scalar.activation(
                                p_sb[:].rearrange("p (i q) -> p i q", i=2)[:, :, d0:],
                                pss[:].rearrange("p (i q) -> p i q", i=2)[:, :, d0:],
                                mybir.ActivationFunctionType.Exp,
                                scale=0.125,
                            )
                        else:
                            nc.scalar.activation(
                                p_sb[:],
                                pss[:],
                                mybir.ActivationFunctionType.Exp,
                                scale=0.125,
                            )
                        if kt >= 4 * qt:  # diagonal tile: causal mask.
                            # one half on DVE, one on Pool: both finish
                            # ~430ns after exp instead of 860ns serial
                            for i, meng in ((0, nc.vector), (1, nc.gpsimd)):
                                meng.tensor_tensor(
                                    p_sb[:, i * 512 + d0 : i * 512 + d0 + 128],
                                    p_sb[:, i * 512 + d0 : i * 512 + d0 + 128],
                                    tri_mask[:],
                                    mybir.AluOpType.mult,
                                )
                        avq.append((kt, p_sb, d0))
                        # AV lags two k-tiles behind exp: by emission time its
                        # exp AND Pool masks have both finished -> no PE wait
                        if len(avq) > 2:
                            emit_av(*avq.pop(0))
                        elif kt == 0 and pending is not None:
                            normalize_pair(*pending)
                            pending = None
                            if qt == QT_TILES - 1 and hj == 1:
                                weave.push(outproj_half_gen(qt, 0))
                        weave.take(2 if len(avq) <= 2 else 1)
                    for a in avq:
                        emit_av(*a)
                    pending = (hj, qt, pso_e, pso_o)

            # ==== block 0: j=0 k/q + v eagerly, then attention on heads 0/1
            # with the j=1 projections (then block 1's) woven in.
            for _ in chain(kq_gen(0, 0), v_gen(0)):
                pass
            for blk in range(QT_TILES):
                gens = []
                if blk == 0:
                    gens.append(kq_gen(0, 1))
                if blk > 0:
                    gens.append(outproj_gen(blk - 1))
                if blk < QT_TILES - 1:
                    gens.append(proj_gen(blk + 1))
                weave = Weave(*gens)
                attn(blk, weave)
                weave.drain()
            normalize_pair(*pending)
            for _ in outproj_half_gen(QT_TILES - 1, 1):
                pass

    nc.compile()
    return nc


_NC_CACHE = None


def _get_program():
    global _NC_CACHE
    if _NC_CACHE is None:
        _NC_CACHE = _build_program()
    return _NC_CACHE


def _run(in_maps, trace=False, **kw):
    nc = _get_program()
    return run_bass_kernel_spmd(nc, in_maps, list(range(NCORES)), trace=trace, **kw)


def _chunked_T(a, nch):
    """[R, Cc] -> [128, nch, Cc] with [p, i, c] = a[i*128+p, c], bf16."""
    r, c = a.shape
    assert r == nch * 128
    return np.ascontiguousarray(
        a.reshape(nch, 128, c).transpose(1, 0, 2)
    ).astype(ml_dtypes.bfloat16)


def _jmajor_T(a):
    """[D, C=256] -> [128, 2, DCH, 128] with [p, j, i, c] = a[i*128+p, j*128+c]."""
    d, c = a.shape
    assert d == DCH * 128 and c == 256
    return np.ascontiguousarray(
        a.reshape(DCH, 128, 2, 128).transpose(1, 2, 0, 3)
    ).astype(ml_dtypes.bfloat16)


def _make_in_maps(x, Wq, bq, Wk, bk, Wv, bv, Wo, bo):
    x = np.asarray(x, dtype=np.float32)
    Wq, Wk, Wv, Wo = (np.asarray(w, dtype=np.float32) for w in (Wq, Wk, Wv, Wo))
    bq, bk, bv = (np.asarray(b, dtype=np.float32) for b in (bq, bk, bv))
    in_maps = []
    xts = [_chunked_T(x[b].T, DCH) for b in range(B)]
    for core in range(NCORES):
        b, g = divmod(core, 4)
        s = slice(g * C, (g + 1) * C)
        b6 = np.stack(
            [
                bq[s][0:128], bq[s][128:256],
                bk[s][0:128], bk[s][128:256],
                bv[s][0:128], bv[s][128:256],
            ],
            axis=1,
        )
        in_maps.append(
            {
                "xt": xts[b],
                "wqt": _jmajor_T(Wq[s, :].T),
                "wkt": _jmajor_T(Wk[s, :].T),
                "wvt": _jmajor_T(Wv[s, :].T),
                "wot": _chunked_T(Wo[:, s].T, 2),
                "b6": np.ascontiguousarray(b6, dtype=np.float32),
            }
        )
    return in_maps


def kernel(x, Wq, bq, Wk, bk, Wv, bv, Wo, bo, _trace=False, _trace_out=None, _tmpdir=None):
    in_maps = _make_in_maps(x, Wq, bq, Wk, bk, Wv, bv, Wo, bo)
    res = _run(in_maps, trace=_trace, tmpdir=_tmpdir)
    if _trace_out is not None:
        _trace_out.append(res)
    bo = np.asarray(bo, dtype=np.float32)
    out = np.empty((B, L, D), dtype=np.float32)
    for b in range(B):
        acc = res.results[4 * b]["out"].astype(np.float32)
        for g in range(1, 4):
            acc = acc + res.results[4 * b + g]["out"].astype(np.float32)
        out[b] = acc + bo[None, :]
    return out


# revision 34
# speedup vs baseline: 1.2116x; 1.0042x over previous
"""Multi-head causal self-attention (B=2, L=2048, D=1024, H=16) on 8 TRN2
NeuronCores.  ~170 us HW exec (v1 baseline 210-214 us).

Sharding: core c handles batch b = c // 4 and head group g = c % 4 (4 heads,
i.e. a 256-wide slice of the QKV output dim and the matching 256 rows of
Wo^T).  Each core computes a full (L, D) partial of the output projection;
the host sums the 4 partials per batch (bf16 on the wire) and adds bo.

Structure:
 * Host pre-transposes + pre-casts x / W slices to bf16 in the exact on-chip
   layout (XT [128,8,2048], W*T [128,2,8,128] j-major, WoT [128,2,1024]) --
   no on-chip transposes/casts; the old 43us of Pool casts + PE/XBAR
   transposes and the staged f32 weight loads are gone entirely.
 * Startup: the ~2.5 MB gating block-0 projections is split across BOTH
   HWDGE rings (sync: wk_j0 + x chunks; scalar: wq_j0 + biases + x chunks +
   wv) in consumption order; the k-projection streams as chunks land.
 * Attention per (qt, head-pair hj, k-tile kt): the two heads' score
   matmuls (K=128, k^T zero-padded per head -- 64-row packed tiles measured
   to re-throttle the PE HAM clock mid-kernel) write a 2-bank PSUM pair;
   ONE exp per k-tile covers both heads (diag tiles use a strided AP to
   skip fully-masked leading cols of both halves AND trim the score
   matmuls themselves); causal mask via Pool multiply with a triangle;
   AV lags TWO k-tiles behind exp so its exp + masks are done by PE issue
   time (lag 3 measured worse).  Denominator via the ones-column of Vp.
 * qk/v projections of block b+1 and the output projection of block b-1
   are generators, woven 1-2 matmuls per attention iteration into the PE
   slack under the 1146ns exp pace; leftovers drain at block ends.  Block
   0 runs kq_j0+v eagerly and weaves its own j=1 half into heads 0/1.
 * normalize = PE ones-broadcast of the denominator row + DVE
   reciprocal_approx_fast + multiply, deferred one head-pair.  Last
   block's output projection split by contraction half (cj0 woven into
   the final pairs, cj1 + bf16 adds + dual-ring stores in the tail).

Measured NOT to work: 64-row tile_position-packed score matmuls (PE runs
at 1.2 GHz while fully busy -- HAM only counts full-128-partition
streams); fp8 q/k projections (rel err 2.6e-2 > 2e-2 gate, numpy sim);
fp8 p (exp overflows e4m3 range); a [2,128] float32r ones tile for a
fused 2-head normalize broadcast (NEFF compile failure); AV lag 3
(+32 us); splitting the first wk DMA into chunks (+2 us, delays x);
draining leftover weave after the final normalize (+1.4 us).
"""

import sys

for _p in ("/opt/trn_rl_repo", "/root/.axon_site/_ro/trn_rl_repo"):
    if _p not in sys.path:
        sys.path.append(_p)

from contextlib import ExitStack

import numpy as np
import ml_dtypes

import concourse.bass as bass
import concourse.tile as tile
from concourse import bacc, mybir
from concourse.bass_utils import run_bass_kernel_spmd
from concourse.masks import make_identity

F32 = mybir.dt.float32
F16 = mybir.dt.float16
BF16 = mybir.dt.bfloat16

B, L, D, H = 2, 2048, 1024, 16
DK = D // H  # 64
NCORES = 8
GH = 4  # heads per core
C = GH * DK  # 256: per-core slice of the qkv/head dim
QT_TILES = L // 512  # 4
DCH = D // 128  # 8


class Weave:
    """FIFO of generators; take(n) advances up to n emission steps."""

    def __init__(self, *gens):
        self.gens = list(gens)

    def push(self, gen):
        self.gens.append(gen)

    def take(self, n):
        while n > 0 and self.gens:
            try:
                next(self.gens[0])
                n -= 1
            except StopIteration:
                self.gens.pop(0)

    def drain(self):
        while self.gens:
            try:
                next(self.gens[0])
            except StopIteration:
                self.gens.pop(0)

    def finish_first(self):
        if self.gens:
            for _ in self.gens[0]:
                pass
            self.gens.pop(0)


def _build_program():
    nc = bacc.Bacc("TRN2", target_bir_lowering=False, debug=False, num_devices=NCORES)

    xt_d = nc.dram_tensor("xt", [128, DCH, L], BF16, kind="ExternalInput").ap()
    wq_d = nc.dram_tensor("wqt", [128, 2, DCH, 128], BF16, kind="ExternalInput").ap()
    wk_d = nc.dram_tensor("wkt", [128, 2, DCH, 128], BF16, kind="ExternalInput").ap()
    wv_d = nc.dram_tensor("wvt", [128, 2, DCH, 128], BF16, kind="ExternalInput").ap()
    wo_d = nc.dram_tensor("wot", [128, 2, D], BF16, kind="ExternalInput").ap()
    b6_d = nc.dram_tensor("b6", [128, 6], F32, kind="ExternalInput").ap()
    out_d = nc.dram_tensor("out", [L, D], BF16, kind="ExternalOutput").ap()

    with tile.TileContext(nc) as tc, ExitStack() as ctx:
        pool = ctx.enter_context(tc.tile_pool(name="persist", bufs=1))
        psum = ctx.enter_context(tc.tile_pool(name="psum", bufs=2, space="PSUM"))
        psum_o = ctx.enter_context(tc.tile_pool(name="psum_o", bufs=2, space="PSUM"))
        psum2 = ctx.enter_context(tc.tile_pool(name="psum2", bufs=2, space="PSUM"))
        cp = ctx.enter_context(tc.tile_pool(name="copies", bufs=3))
        yp = ctx.enter_context(tc.tile_pool(name="youts", bufs=2))

        XT = pool.tile([128, DCH, L], BF16)
        WT = {}
        for name in ("q", "k", "v"):
            WT[name] = pool.tile([128, 2, DCH, 128], BF16, name=f"W{name}T")
        WoT = pool.tile([128, 2, D], BF16)
        b6_sb = pool.tile([128, 6], F32)

        # ---- prologue DMAs.  The ~2.5 MB that gates block-0's projections
        # (wk/wq j=0 halves, x block 0, wv) is split evenly across the two
        # HWDGE rings in consumption order so the k/q/v matmul streams start
        # as chunks land; everything later-needed follows.
        nc.sync.dma_start(WT["k"][:, 0], wk_d[:, 0])
        for dci in range(0, 4):
            nc.sync.dma_start(XT[:, dci, 0:512], xt_d[:, dci, 0:512])
        nc.scalar.dma_start(WT["q"][:, 0], wq_d[:, 0])
        nc.scalar.dma_start(b6_sb[:], b6_d[:])
        for dci in range(4, 8):
            nc.scalar.dma_start(XT[:, dci, 0:512], xt_d[:, dci, 0:512])
        nc.sync.dma_start(WT["v"][:, 0], wv_d[:, 0])
        nc.sync.dma_start(WT["v"][:, 1], wv_d[:, 1])
        nc.scalar.dma_start(WT["k"][:, 1], wk_d[:, 1])
        nc.scalar.dma_start(WT["q"][:, 1], wq_d[:, 1])
        for dci in range(0, 4):
            nc.sync.dma_start(XT[:, dci, 512:1024], xt_d[:, dci, 512:1024])
        for dci in range(4, 8):
            nc.scalar.dma_start(XT[:, dci, 512:1024], xt_d[:, dci, 512:1024])
        nc.sync.dma_start(WoT[:], wo_d[:])
        for dci in range(DCH):
            nc.sync.dma_start(XT[:, dci, 1024:2048], xt_d[:, dci, 1024:2048])

        BIAS_COL = {"q": 0, "k": 2, "v": 4}

        ones_f32 = pool.tile([1, 128], F32)
        nc.gpsimd.memset(ones_f32[:], 1.0)
        ones_r = pool.tile([1, 128], mybir.dt.float32r)
        nc.vector.tensor_copy(ones_r[:], ones_f32[:])
        ident16 = pool.tile([128, 128], F16)
        make_identity(nc, ident16)
        tri_mask = pool.tile([128, 128], F16)
        nc.gpsimd.memset(tri_mask[:], 1.0)
        nc.gpsimd.affine_select(
            out=tri_mask[:],
            in_=tri_mask[:],
            pattern=[[1, 128]],
            compare_op=mybir.AluOpType.is_ge,
            fill=0.0,
            base=0,
            channel_multiplier=-1,
        )

        QTs = [pool.tile([128, 2, 512], F16, name=f"QT{g}") for g in range(4)]
        # k^T zero-padded per head to K=128 rows: the PE HAM clock gate only
        # un-throttles when matmuls stream all 128 partitions (64-row packed
        # score tiles measured to re-throttle the clock mid-kernel).
        KTzs = [pool.tile([128, GH, 512], F16, name=f"KTz{g}") for g in range(4)]
        Vp16 = [pool.tile([128, 4, GH, DK + 1], F16, name=f"Vp16_{g}") for g in range(4)]
        OTs = [pool.tile([128, 2, 512], BF16, name=f"OT{g}") for g in range(4)]

        for g in range(4):
            for h in range(GH):
                zp = 64 - 64 * (h % 2)
                nc.gpsimd.memset(KTzs[g][zp : zp + 64, h, :], 0.0)
        for g in range(4):
            nc.gpsimd.memset(Vp16[g][:, :, :, DK], 1.0)

        with nc.allow_low_precision(reason="bf16/f16 matmul inputs"):

            def kq_gen(blk, j):
                # k then q projection of one 128-col slice; yields per PE op.
                for name in ("k", "q"):
                    ps = psum.tile([128, 512], F32, tag="ps")
                    for dci in range(DCH):
                        nc.tensor.matmul(
                            ps[:],
                            lhsT=WT[name][:, j, dci, :],
                            rhs=XT[:, dci, blk * 512 : (blk + 1) * 512],
                            start=(dci == 0),
                            stop=(dci == DCH - 1),
                        )
                        yield
                    if name == "q":
                        nc.vector.tensor_tensor(
                            QTs[blk][:, j, :],
                            ps[:],
                            b6_sb[:, BIAS_COL["q"] + j, None].to_broadcast(
                                (128, 512)
                            ),
                            mybir.AluOpType.add,
                        )
                    else:
                        for half in range(2):
                            hp = 64 * half
                            nc.vector.tensor_tensor(
                                KTzs[blk][hp : hp + 64, 2 * j + half, :],
                                ps[hp : hp + 64, :],
                                b6_sb[
                                    hp : hp + 64, BIAS_COL["k"] + j, None
                                ].to_broadcast((64, 512)),
                                mybir.AluOpType.add,
                            )
                    yield

            def v_gen(blk):
                vt = cp.tile([128, 2, 512], F16, tag="vt", bufs=2)
                for ch in range(2):
                    ps = psum.tile([128, 512], F32, tag="ps")
                    for dci in range(DCH):
                        nc.tensor.matmul(
                            ps[:],
                            lhsT=WT["v"][:, ch, dci, :],
                            rhs=XT[:, dci, blk * 512 : (blk + 1) * 512],
                            start=(dci == 0),
                            stop=(dci == DCH - 1),
                        )
                        yield
                    nc.vector.tensor_tensor(
                        vt[:, ch, :],
                        ps[:],
                        b6_sb[:, BIAS_COL["v"] + ch, None].to_broadcast((128, 512)),
                        mybir.AluOpType.add,
                    )
                    yield
                for lsub in range(4):
                    pv = psum.tile([128, 256], F16, tag="ps")
                    for ch in range(2):
                        nc.tensor.matmul(
                            pv[:, ch * 128 : (ch + 1) * 128],
                            lhsT=vt[:, ch, lsub * 128 : (lsub + 1) * 128],
                            rhs=ident16[:],
                            is_transpose=True,
                            start=(ch == 0),
                            stop=(ch == 1),
                        )
                        yield
                    nc.vector.tensor_copy(
                        Vp16[blk][:, lsub, :, 0:DK],
                        pv[:].rearrange("p (h d) -> p h d", h=GH),
                    )
                    yield

            def chain(*gens):
                for g in gens:
                    yield from g

            def proj_gen(blk):
                # j=0 k/q first, then v, then j=1: attention on the block can
                # begin (heads 0/1) before the j=1 half exists.
                return chain(kq_gen(blk, 0), v_gen(blk), kq_gen(blk, 1))

            def normalize(h, qt, pso):
                hj, hp = h // 2, 64 * (h % 2)
                den_r = cp.tile([1, 512], mybir.dt.float32r, tag="den", bufs=2)
                nc.vector.tensor_copy(den_r[:], pso[64:65, :])
                psb = psum.tile([128, 512], F32, tag="ps")
                nc.tensor.matmul(
                    psb[:64], lhsT=ones_r[:, 0:64], rhs=den_r[:], start=True, stop=True
                )
                rb = cp.tile([64, 512], F32, tag="rb", bufs=2)
                nc.vector.reciprocal_approx_fast(rb[:], psb[:64])
                nc.vector.tensor_tensor(
                    OTs[qt][hp : hp + 64, hj, :],
                    pso[:64],
                    rb[:],
                    mybir.AluOpType.mult,
                )

            def normalize_pair(hj, qt, pso_e, pso_o):
                normalize(2 * hj, qt, pso_e)
                normalize(2 * hj + 1, qt, pso_o)

            def outproj_gen(qt512):
                for sub in range(4):
                    for e in range(2):
                        psy = psum.tile([128, 512], F32, tag="ps")
                        for cj in range(2):
                            nc.tensor.matmul(
                                psy[:],
                                lhsT=OTs[qt512][:, cj, sub * 128 : (sub + 1) * 128],
                                rhs=WoT[:, cj, e * 512 : (e + 1) * 512],
                                start=(cj == 0),
                                stop=(cj == 1),
                            )
                            yield
                        y_sb = yp.tile([128, 512], BF16, tag="y")
                        nc.vector.tensor_copy(y_sb[:], psy[:])
                        q0 = qt512 * 512 + sub * 128
                        nc.sync.dma_start(
                            out_d[q0 : q0 + 128, e * 512 : (e + 1) * 512], y_sb[:]
                        )
                        yield

            y3 = {}
            y8 = ctx.enter_context(tc.tile_pool(name="ytail", bufs=8))

            def outproj_half_gen(qt512, cj, subs=(0, 1, 2, 3)):
                # last-block outproj split by contraction half: cj=0 runs
                # during the final attention pair, cj=1 + accumulate + DMA is
                # all that trails the final normalize.
                for sub in subs:
                    for e in range(2):
                        idx = sub * 2 + e
                        psy = psum.tile([128, 512], F32, tag="ps")
                        nc.tensor.matmul(
                            psy[:],
                            lhsT=OTs[qt512][:, cj, sub * 128 : (sub + 1) * 128],
                            rhs=WoT[:, cj, e * 512 : (e + 1) * 512],
                            start=True,
                            stop=True,
                        )
                        yield
                        if cj == 0:
                            y_sb = y8.tile([128, 512], F32, tag="y8")
                            nc.vector.tensor_copy(y_sb[:], psy[:])
                            y3[idx] = y_sb
                        else:
                            y_sb = y3[idx]
                            yb = y8.tile([128, 512], BF16, tag="yb8")
                            nc.vector.tensor_tensor(
                                yb[:], y_sb[:], psy[:], mybir.AluOpType.add
                            )
                            q0 = qt512 * 512 + sub * 128
                            eng = nc.sync if idx % 2 == 0 else nc.scalar
                            eng.dma_start(
                                out_d[q0 : q0 + 128, e * 512 : (e + 1) * 512], yb[:]
                            )
                        yield

            pending = None

            def attn(qt, weave):
                n_kt = 4 * qt + 4
                avq = []

                def make_pair(hj):
                    pso_e = psum_o.tile([128, 512], F32, tag="pso")
                    pso_o = psum_o.tile([128, 512], F32, tag="pso")

                    def emit_av(kt, p_sb, d0):
                        g, ksub = kt // 4, kt % 4
                        for i, pso in ((0, pso_e), (1, pso_o)):
                            nc.tensor.matmul(
                                pso[:65, d0:],
                                lhsT=Vp16[g][:, ksub, 2 * hj + i, :],
                                rhs=p_sb[:, i * 512 + d0 : (i + 1) * 512],
                                start=(kt == 0),
                                stop=(kt == n_kt - 1),
                                skip_group_check=True,
                            )
                        if kt == n_kt - 1:  # pair complete: normalize now
                            normalize_pair(hj, qt, pso_e, pso_o)
                            if qt == QT_TILES - 1 and hj == 0:
                                weave.push(outproj_half_gen(qt, 0))

                    return emit_av

                for hj in range(2):
                    if qt == 0 and hj == 1:
                        # block-0 only: finish j=1 projections before heads 2/3
                        weave.finish_first()
                    emit_av = make_pair(hj)
                    for kt in range(n_kt):
                        g, ksub = kt // 4, kt % 4
                        pss = psum2.tile([128, 1024], F32, tag="ps2")
                        d0 = max(0, (kt - 4 * qt) * 128)
                        for i in range(2):  # the head pair, full-K=128 each
                            nc.tensor.matmul(
                                pss[:, i * 512 + d0 : (i + 1) * 512],
                                lhsT=KTzs[g][
                                    :, 2 * hj + i, ksub * 128 : (ksub + 1) * 128
                                ],
                                rhs=QTs[qt][:, hj, d0:],
                                start=True,
                                stop=True,
                            )
                        p_sb = cp.tile([128, 1024], F16, tag="p", bufs=5)
                        if d0:
                            nc.scalar.activation(
                                p_sb[:].rearrange("p (i q) -> p i q", i=2)[:, :, d0:],
                                pss[:].rearrange("p (i q) -> p i q", i=2)[:, :, d0:],
                                mybir.ActivationFunctionType.Exp,
                                scale=0.125,
                            )
                        else:
                            nc.scalar.activation(
                                p_sb[:],
                                pss[:],
                                mybir.ActivationFunctionType.Exp,
                                scale=0.125,
                            )
                        if kt >= 4 * qt:  # diagonal tile: causal mask.
                            # one half on DVE, one on Pool: both finish
                            # ~430ns after exp instead of 860ns serial
                            for i, meng in ((0, nc.vector), (1, nc.gpsimd)):
                                meng.tensor_tensor(
                                    p_sb[:, i * 512 + d0 : i * 512 + d0 + 128],
                                    p_sb[:, i * 512 + d0 : i * 512 + d0 + 128],
                                    tri_mask[:],
                                    mybir.AluOpType.mult,
                                )
                        # AV lags two k-tiles behind exp, ACROSS pair
                        # boundaries: the previous pair's trailing AVs
                        # interleave with this pair's first scores instead
                        # of stalling on their exps
                        avq.append((emit_av, kt, p_sb, d0))
                        if len(avq) > 2:
                            fn, k2, pb, dd = avq.pop(0)
                            fn(k2, pb, dd)
                        weave.take(2 if len(avq) <= 2 else 1)
                for fn, k2, pb, dd in avq:
                    fn(k2, pb, dd)

            # ==== block 0: j=0 k/q + v eagerly, then attention on heads 0/1
            # with the j=1 projections (then block 1's) woven in.
            for _ in chain(kq_gen(0, 0), v_gen(0)):
                pass
            for blk in range(QT_TILES):
                gens = []
                if blk == 0:
                    gens.append(kq_gen(0, 1))
                if blk > 0:
                    gens.append(outproj_gen(blk - 1))
                if blk < QT_TILES - 1:
                    gens.append(proj_gen(blk + 1))
                weave = Weave(*gens)
                attn(blk, weave)
                weave.drain()
            for _ in outproj_half_gen(QT_TILES - 1, 1):
                pass

    nc.compile()
    return nc


_NC_CACHE = None


def _get_program():
    global _NC_CACHE
    if _NC_CACHE is None:
        _NC_CACHE = _build_program()
    return _NC_CACHE


def _run(in_maps, trace=False, **kw):
    nc = _get_program()
    return run_bass_kernel_spmd(nc, in_maps, list(range(NCORES)), trace=trace, **kw)


def _chunked_T(a, nch):
    """[R, Cc] -> [128, nch, Cc] with [p, i, c] = a[i*128+p, c], bf16."""
    r, c = a.shape
    assert r == nch * 128
    return np.ascontiguousarray(
        a.reshape(nch, 128, c).transpose(1, 0, 2)
    ).astype(ml_dtypes.bfloat16)


def _jmajor_T(a):
    """[D, C=256] -> [128, 2, DCH, 128] with [p, j, i, c] = a[i*128+p, j*128+c]."""
    d, c = a.shape
    assert d == DCH * 128 and c == 256
    return np.ascontiguousarray(
        a.reshape(DCH, 128, 2, 128).transpose(1, 2, 0, 3)
    ).astype(ml_dtypes.bfloat16)


def _make_in_maps(x, Wq, bq, Wk, bk, Wv, bv, Wo, bo):
    x = np.asarray(x, dtype=np.float32)
    Wq, Wk, Wv, Wo = (np.asarray(w, dtype=np.float32) for w in (Wq, Wk, Wv, Wo))
    bq, bk, bv = (np.asarray(b, dtype=np.float32) for b in (bq, bk, bv))
    in_maps = []
    xts = [_chunked_T(x[b].T, DCH) for b in range(B)]
    for core in range(NCORES):
        b, g = divmod(core, 4)
        s = slice(g * C, (g + 1) * C)
        b6 = np.stack(
            [
                bq[s][0:128], bq[s][128:256],
                bk[s][0:128], bk[s][128:256],
                bv[s][0:128], bv[s][128:256],
            ],
            axis=1,
        )
        in_maps.append(
            {
                "xt": xts[b],
                "wqt": _jmajor_T(Wq[s, :].T),
                "wkt": _jmajor_T(Wk[s, :].T),
                "wvt": _jmajor_T(Wv[s, :].T),
                "wot": _chunked_T(Wo[:, s].T, 2),
                "b6": np.ascontiguousarray(b6, dtype=np.float32),
            }
        )
    return in_maps


def kernel(x, Wq, bq, Wk, bk, Wv, bv, Wo, bo, _trace=False, _trace_out=None, _tmpdir=None):
    in_maps = _make_in_maps(x, Wq, bq, Wk, bk, Wv, bv, Wo, bo)
    res = _run(in_maps, trace=_trace, tmpdir=_tmpdir)
    if _trace_out is not None:
        _trace_out.append(res)
    bo = np.asarray(bo, dtype=np.float32)
    out = np.empty((B, L, D), dtype=np.float32)
    for b in range(B):
        acc = res.results[4 * b]["out"].astype(np.float32)
        for g in range(1, 4):
            acc = acc + res.results[4 * b + g]["out"].astype(np.float32)
        out[b] = acc + bo[None, :]
    return out
